# revision 1
# baseline (speedup 1.0000x reference)
"""Trainium2 Bass kernel for the binarized MLP (BNN) problem.

Network (eval mode):
  h1 = sign(bn1(x @ sign(w1).T + b1))        x: [8192, 784]
  h2 = sign(bn2(h1 @ sign(w2).T + b2))       hidden: 6144
  h3 = sign(bn3(h2 @ sign(w3).T + b3))
  out = log_softmax(h3 @ w4.T + b4)          out: [8192, 10]
(clip(-1,1) before sign does not change sign, so it is dropped.)

Strategy:
  * Data-parallel over the batch: 8 cores x 1024 rows, no collectives.
  * All activations live transposed in SBUF as hT[H, B] so each layer's
    output feeds the next layer's matmul rhs directly (zero transposes).
  * BN + bias + clip + binarize folds to sign(h*s + c) with
    s = g*rsqrt(v+eps), c = (b - m)*s + be  -> one scalar-engine
    activation (Sign) per psum tile with per-partition scale/bias.
  * Layer 1 (real-valued x, contraction 784->pad 896): x is split into
    2 fp16 planes (hi/lo) so the fp16 matmuls reproduce fp32 precision
    (residual 2^-23; the PE keeps fp16 denormals, verified on HW);
    weights are exact +-1 in fp16.
  * Layers 2/3 (+-1 x +-1, contraction 6144): fp8e4m3 with DoubleRow
    perf mode - products and fp32 PSUM accumulation are exact.
  * Layer 4: w4 (split hi/lo bf16, stacked at psum partitions 0-9 and
    32-41) is the stationary operand; h3 streams as fp8 rhs (mixed-dtype
    matmul, exact) giving logitsT; hi+lo rows summed + fp32 b4 on DVE, PE
    transposes [10,128] blocks back and log_softmax runs per [128, 10]
    tile (reduce_max, Exp with accumulate, Ln, subtract).
"""

import numpy as np
import ml_dtypes

H = 6144
B_TOTAL = 8192
N_CORES = 8
B = B_TOTAL // N_CORES  # 1024 rows per core
K1 = 784
K1P = 896  # 7 * 128
EPS = 1e-5
P = 128
M_TILES = H // P  # 48
NB = B // 512  # psum-width chunks per core
BCH = B // P  # 8 output row-chunks per core

_BF16 = ml_dtypes.bfloat16
_FP8 = ml_dtypes.float8_e4m3
_FP16 = np.float16


def _binarize(w):
    return np.where(w >= 0, np.float32(1.0), np.float32(-1.0))


def _pack_weight(wb, kpad, dtype):
    """[Hout, K] +-1 matrix -> [Hout/128, 128, kpad/128, 128] tiles where
    pack[m, p, ko, j] = wb[m*128 + j, ko*128 + p] (lhsT layout)."""
    hout, k = wb.shape
    if k < kpad:
        wb = np.concatenate([wb, np.zeros((hout, kpad - k), np.float32)], axis=1)
    return np.ascontiguousarray(
        wb.reshape(hout // P, P, kpad // P, P).transpose(0, 3, 2, 1)
    ).astype(dtype)


def _pack_rhs(xc):
    """[B, K] -> [128, K/128, B] with pack[p, ko, b] = xc[b, ko*128+p]."""
    b, k = xc.shape
    return np.ascontiguousarray(xc.T.reshape(k // P, P, b).transpose(1, 0, 2))


def build_nc():
    """Build the (single-program, run-on-8-cores) Bass kernel."""
    import concourse.tile as tile
    import concourse.mybir as mybir
    from concourse import bacc
    from concourse.masks import make_identity

    af = mybir.ActivationFunctionType
    f32 = mybir.dt.float32
    bf16 = mybir.dt.bfloat16
    f16 = mybir.dt.float16
    f8 = mybir.dt.float8e4

    nc = bacc.Bacc(
        "TRN2",
        target_bir_lowering=False,
        debug=False,
        enable_asserts=False,
        num_devices=N_CORES,
    )

    t = {}
    for nm in ("xhi", "xlo"):
        t[nm] = nc.dram_tensor(nm, [P, K1P // P, B], f16, kind="ExternalInput").ap()
    t["w1p"] = nc.dram_tensor(
        "w1p", [M_TILES, P, K1P // P, P], f16, kind="ExternalInput"
    ).ap()
    for nm in ("w2p", "w3p"):
        t[nm] = nc.dram_tensor(
            nm, [M_TILES, P, M_TILES, P], f8, kind="ExternalInput"
        ).ap()
    t["w4s"] = nc.dram_tensor(
        "w4s", [M_TILES, P, 42], bf16, kind="ExternalInput"
    ).ap()
    t["b4t"] = nc.dram_tensor("b4t", [10, 1], f32, kind="ExternalInput").ap()
    for i in (1, 2, 3):
        t[f"s{i}"] = nc.dram_tensor(f"s{i}", [P, M_TILES], f32, kind="ExternalInput").ap()
        t[f"c{i}"] = nc.dram_tensor(f"c{i}", [P, M_TILES], f32, kind="ExternalInput").ap()
    t["out"] = nc.dram_tensor("out", [B, 10], f32, kind="ExternalOutput").ap()

    from contextlib import ExitStack

    with tile.TileContext(nc) as tc, ExitStack() as ctx:
        consts = ctx.enter_context(tc.tile_pool(name="consts", bufs=1))
        xpool = ctx.enter_context(tc.tile_pool(name="x", bufs=1))
        hpool = ctx.enter_context(tc.tile_pool(name="h", bufs=2))
        w1pool = ctx.enter_context(tc.tile_pool(name="w1", bufs=3))
        wpool = ctx.enter_context(tc.tile_pool(name="w", bufs=4))
        pspool = ctx.enter_context(tc.tile_pool(name="ps", bufs=3, space="PSUM"))
        ps4pool = ctx.enter_context(tc.tile_pool(name="ps4", bufs=2, space="PSUM"))
        ps20pool = ctx.enter_context(tc.tile_pool(name="ps20", bufs=2, space="PSUM"))
        small = ctx.enter_context(tc.tile_pool(name="small", bufs=4))

        # ---- one-time loads ----
        xhi_t = xpool.tile([P, K1P // P, B], f16, tag="xhi")
        xlo_t = xpool.tile([P, K1P // P, B], f16, tag="xlo")
        nc.sync.dma_start(xhi_t[:, 0, :], t["xhi"][:, 0, :])
        nc.gpsimd.dma_start(xlo_t[:, 0, :], t["xlo"][:, 0, :])
        wt0 = w1pool.tile([P, K1P // P, P], f16, tag="w1")
        nc.sync.dma_start(wt0[:], t["w1p"][0])
        for k in range(1, K1P // P):
            nc.sync.dma_start(xhi_t[:, k, :], t["xhi"][:, k, :])
            nc.gpsimd.dma_start(xlo_t[:, k, :], t["xlo"][:, k, :])
        xs = [xhi_t, xlo_t]
        bn = []
        for i in (1, 2, 3):
            s_t = consts.tile([P, M_TILES], f32, tag=f"s{i}")
            nc.gpsimd.dma_start(s_t[:], t[f"s{i}"][:])
            c_t = consts.tile([P, M_TILES], f32, tag=f"c{i}")
            nc.gpsimd.dma_start(c_t[:], t[f"c{i}"][:])
            bn.append((s_t, c_t))
        w4sb = consts.tile([P, M_TILES, 42], bf16, tag="w4")
        nc.gpsimd.dma_start(w4sb[:], t["w4s"].rearrange("k p n -> p k n"))
        b4sb = consts.tile([10, 1], f32, tag="b4")
        nc.gpsimd.dma_start(b4sb[:], t["b4t"][:])
        ident10 = consts.tile([10, 10], f32, tag="ident")
        make_identity(nc, ident10[:])

        # ---- layer 1: 2-way fp16 split of x, K = 896 ----
        s_t, c_t = bn[0]
        h1 = hpool.tile([P, M_TILES, B], f8, tag="h")
        for m in range(M_TILES):
            if m == 0:
                wt = wt0
            else:
                wt = w1pool.tile([P, K1P // P, P], f16, tag="w1")
                nc.sync.dma_start(wt[:], t["w1p"][m])
            for n in range(NB):
                ps = pspool.tile([P, 512], f32, tag="ps")
                for k in range(K1P // P):
                    for si, xt in enumerate(xs):
                        nc.tensor.matmul(
                            ps[:],
                            wt[:, k, :],
                            xt[:, k, n * 512 : (n + 1) * 512],
                            start=(k == 0 and si == 0),
                            stop=(k == K1P // P - 1 and si == len(xs) - 1),
                        )
                nc.scalar.activation(
                    h1[:, m, n * 512 : (n + 1) * 512],
                    ps[:],
                    af.Sign,
                    bias=c_t[:, m : m + 1],
                    scale=s_t[:, m : m + 1],
                )

        # ---- layers 2 and 3: exact +-1 fp8 DoubleRow matmuls ----
        hin = h1
        for li, wname in ((1, "w2p"), (2, "w3p")):
            s_t, c_t = bn[li]
            hout = hpool.tile([P, M_TILES, B], f8, tag="h")
            for m in range(M_TILES):
                wt = wpool.tile([P, M_TILES, P], f8, tag="w")
                (nc.sync if m % 2 == 0 else nc.gpsimd).dma_start(wt[:], t[wname][m])
                for n in range(NB):
                    ps = pspool.tile([P, 512], f32, tag="ps")
                    for k2 in range(M_TILES // 2):
                        nc.tensor.matmul(
                            ps[:],
                            wt[:, 2 * k2 : 2 * k2 + 2, :],
                            hin[:, 2 * k2 : 2 * k2 + 2, n * 512 : (n + 1) * 512],
                            start=(k2 == 0),
                            stop=(k2 == M_TILES // 2 - 1),
                            perf_mode=mybir.MatmulPerfMode.DoubleRow,
                        )
                    nc.scalar.activation(
                        hout[:, m, n * 512 : (n + 1) * 512],
                        ps[:],
                        af.Sign,
                        bias=c_t[:, m : m + 1],
                        scale=s_t[:, m : m + 1],
                    )
            hin = hout
        h3 = hin

        # ---- layer 4 + log_softmax ----
        # Pre-stage the Ln act-table while the L4 matmuls run (all Sign ops
        # are done by now, so nothing evicts it before the tail Ln).
        warm = small.tile([1, 1], f32, tag="warm")
        nc.vector.memset(warm[:], 1.0)
        warmo = small.tile([1, 1], f32, tag="warmo")
        nc.scalar.activation(warmo[:], warm[:], af.Ln)
        se_all = small.tile([P, BCH], f32, tag="se_all")
        otp_all = small.tile([P, BCH, 10], f32, tag="otp_all")
        for n in range(NB):
            ps20 = ps20pool.tile([64, 512], f32, tag="ps20")
            for k in range(M_TILES):
                nc.tensor.matmul(
                    ps20[0:42, :],
                    w4sb[:, k, :],
                    h3[:, k, n * 512 : (n + 1) * 512],
                    start=(k == 0),
                    stop=(k == M_TILES - 1),
                )
            lg = small.tile([10, 512], f32, tag="lg")
            nc.vector.tensor_scalar_add(lg[:], ps20[0:10, :], b4sb[:])
            lgb = small.tile([10, 512], f32, tag="lgb")
            nc.vector.tensor_add(lgb[:], lg[:], ps20[32:42, :])
            # pass A per 128-row block: transpose, max, shift, exp(+sum)
            for bi in range(4):
                pst = ps4pool.tile([P, 10], f32, tag="pst")
                nc.tensor.transpose(pst[:], lgb[:, bi * P : (bi + 1) * P], ident10[:])
                nmx = small.tile([P, 1], f32, tag="nmx")
                nc.vector.reduce_max(nmx[:], pst[:], axis=mybir.AxisListType.X, negate=True)
                ci = n * 4 + bi
                nc.vector.tensor_scalar_add(otp_all[:, ci, :], pst[:], nmx[:])
                ex = small.tile([P, 10], f32, tag="ex")
                nc.scalar.activation(
                    ex[:],
                    pst[:],
                    af.Exp,
                    bias=nmx[:],
                    scale=1.0,
                    accum_out=se_all[:, ci : ci + 1],
                )
        # pass B: one batched Ln, one broadcast subtract, one store
        ls_all = small.tile([P, BCH], f32, tag="ls_all")
        nc.scalar.activation(ls_all[:], se_all[:], af.Ln)
        acc = small.tile([P, BCH, 10], f32, tag="acc")
        nc.vector.tensor_sub(
            acc[:],
            otp_all[:],
            ls_all[:].rearrange("p (b o) -> p b o", o=1).broadcast_to([P, BCH, 10]),
        )
        nc.sync.dma_start(t["out"].rearrange("(b p) n -> p b n", p=P), acc[:])

    nc.compile()
    return nc


def prepare_in_maps(inputs):
    """Host-side packing: binarize weights, fold BN, split/shard x."""
    x = np.asarray(inputs["x"], np.float32).reshape(-1, K1)

    w1p = _pack_weight(_binarize(np.asarray(inputs["w1"], np.float32)), K1P, _FP16)
    w2p = _pack_weight(_binarize(np.asarray(inputs["w2"], np.float32)), H, _FP8)
    w3p = _pack_weight(_binarize(np.asarray(inputs["w3"], np.float32)), H, _FP8)

    # w4 hi/lo split, transposed to [K, 10], stacked hi|lo -> [48, 128, 20]
    w4 = np.asarray(inputs["w4"], np.float32)
    b4 = np.asarray(inputs["b4"], np.float32)
    w4T = np.ascontiguousarray(w4.T)  # [6144, 10]
    w4hi = w4T.astype(_BF16)
    w4lo = (w4T - w4hi.astype(np.float32)).astype(_BF16)
    w4s = np.zeros((M_TILES, P, 42), _BF16)
    w4s[:, :, 0:10] = w4hi.reshape(M_TILES, P, 10)
    w4s[:, :, 32:42] = w4lo.reshape(M_TILES, P, 10)
    b4t = np.ascontiguousarray(b4.reshape(10, 1))

    sc = {}
    for i in (1, 2, 3):
        g = np.asarray(inputs[f"g{i}"], np.float32)
        be = np.asarray(inputs[f"be{i}"], np.float32)
        m = np.asarray(inputs[f"m{i}"], np.float32)
        v = np.asarray(inputs[f"v{i}"], np.float32)
        b = np.asarray(inputs[f"b{i}"], np.float32)
        s = g / np.sqrt(v + np.float32(EPS))
        c = (b - m) * s + be
        sc[f"s{i}"] = np.ascontiguousarray(s.reshape(M_TILES, P).T)
        sc[f"c{i}"] = np.ascontiguousarray(c.reshape(M_TILES, P).T)

    # x: 2-way fp16 split (PE keeps fp16 denormals), pad + shard + pack
    x_hi = x.astype(_FP16)
    x_lo = (x - x_hi.astype(np.float32)).astype(_FP16)

    in_maps = []
    for core in range(N_CORES):
        sl = slice(core * B, (core + 1) * B)
        im = {
            "w1p": w1p,
            "w2p": w2p,
            "w3p": w3p,
            "w4s": w4s,
            "b4t": b4t,
            **sc,
        }
        for nm, arr in (("xhi", x_hi), ("xlo", x_lo)):
            xc = np.zeros((B, K1P), _FP16)
            xc[:, :K1] = arr[sl]
            im[nm] = _pack_rhs(xc)
        in_maps.append(im)
    return in_maps


_NC_CACHE = []


def kernel(**inputs):
    import time

    from concourse.bass_utils import run_bass_kernel_spmd

    if not _NC_CACHE:
        _NC_CACHE.append(build_nc())
    nc = _NC_CACHE[0]

    in_maps = prepare_in_maps(inputs)
    last_err = None
    for attempt in range(3):
        try:
            res = run_bass_kernel_spmd(nc, in_maps, core_ids=list(range(N_CORES)))
            return np.concatenate([r["out"] for r in res.results], axis=0)
        except Exception as e:  # transient device errors (e.g. NRT exec unit)
            last_err = e
            time.sleep(5 * (attempt + 1))
    raise last_err



# revision 27
# speedup vs baseline: 1.0260x; 1.0260x over previous
"""Trainium2 Bass kernel for the binarized MLP (BNN) problem.

Network (eval mode):
  h1 = sign(bn1(x @ sign(w1).T + b1))        x: [8192, 784]
  h2 = sign(bn2(h1 @ sign(w2).T + b2))       hidden: 6144
  h3 = sign(bn3(h2 @ sign(w3).T + b3))
  out = log_softmax(h3 @ w4.T + b4)          out: [8192, 10]
(clip(-1,1) before sign does not change sign, so it is dropped.)

Strategy:
  * Data-parallel over the batch: 8 cores x 1024 rows, no collectives.
  * All activations live transposed in SBUF as hT[H, B] so each layer's
    output feeds the next layer's matmul rhs directly (zero transposes).
  * BN + bias + clip + binarize folds to sign(h*s + c) with
    s = g*rsqrt(v+eps), c = (b - m)*s + be  -> one scalar-engine
    activation (Sign) per psum tile with per-partition scale/bias.
  * Layer 1 (real-valued x, contraction 784): x is split into 2 fp16
    planes (hi/lo) so the fp16 matmuls reproduce fp32 precision
    (residual 2^-23; the PE keeps fp16 denormals, verified on HW);
    weights are exact +-1 in fp16.  The two planes are packed into
    12 full 128-row k-tiles (2*768 rows) plus one 32-row tail tile
    (2*16 rows), so each psum tile needs 13 matmuls instead of 14.
  * Layers 2/3 (+-1 x +-1, contraction 6144): fp8e4m3 with DoubleRow
    perf mode - products and fp32 PSUM accumulation are exact.
  * Layer 4: w4.T is scaled by 64 and decomposed into 4 exact fp8
    planes (e4m3 peel-off), stacked at psum partitions 0/32/64/96 of a
    [106, 512] DoubleRow matmul against the fp8 h3 - halves the
    classifier matmul count vs a bf16 hi/lo scheme.  DVE sums the 4
    plane rows + 64*b4, the PE transposes [10,128] blocks back with a
    (1/64)-scaled identity, and log_softmax runs per [128, 10] tile
    (reduce_max, Exp with accumulate, Ln, subtract).
"""

import numpy as np
import ml_dtypes

H = 6144
B_TOTAL = 8192
N_CORES = 8
B = B_TOTAL // N_CORES  # 1024 rows per core
K1 = 784
EPS = 1e-5
P = 128
M_TILES = H // P  # 48
NB = B // 512  # psum-width chunks per core
BCH = B // P  # 8 output row-chunks per core
KF = 12  # full 128-row k-tiles in layer 1 (2 planes x 768 rows)
KT = 32  # tail k-tile rows (2 planes x 16 rows)
NPL = 4  # fp8 planes for w4
W4M = 112  # psum partitions for layer 4 (planes at 0/32/64/96, padded so the
# DoubleRow weight AP's Ko stride (= W4M fp8 bytes) is 16-byte aligned
W4SCALE = 64.0

_BF16 = ml_dtypes.bfloat16
_FP8 = ml_dtypes.float8_e4m3
_FP16 = np.float16


def _binarize(w):
    return np.where(w >= 0, np.float32(1.0), np.float32(-1.0))


def _pack_weight(wb, kpad, dtype):
    """[Hout, K] +-1 matrix -> [Hout/128, 128, kpad/128, 128] tiles where
    pack[m, p, ko, j] = wb[m*128 + j, ko*128 + p] (lhsT layout)."""
    hout, k = wb.shape
    if k < kpad:
        wb = np.concatenate([wb, np.zeros((hout, kpad - k), np.float32)], axis=1)
    return np.ascontiguousarray(
        wb.reshape(hout // P, P, kpad // P, P).transpose(0, 3, 2, 1)
    ).astype(dtype)


def build_nc():
    """Build the (single-program, run-on-8-cores) Bass kernel."""
    import concourse.tile as tile
    import concourse.mybir as mybir
    from concourse import bacc
    from concourse.masks import make_identity

    af = mybir.ActivationFunctionType
    f32 = mybir.dt.float32
    f16 = mybir.dt.float16
    f8 = mybir.dt.float8e4

    nc = bacc.Bacc(
        "TRN2",
        target_bir_lowering=False,
        debug=False,
        enable_asserts=False,
        num_devices=N_CORES,
    )

    t = {}
    t["xp"] = nc.dram_tensor("xp", [P, KF, B], f16, kind="ExternalInput").ap()
    t["xt"] = nc.dram_tensor("xt", [P, B], f16, kind="ExternalInput").ap()
    t["w1f"] = nc.dram_tensor(
        "w1f", [M_TILES, P, KF // 2, P], f16, kind="ExternalInput"
    ).ap()
    t["w1t"] = nc.dram_tensor(
        "w1t", [M_TILES // 2, P, P], f16, kind="ExternalInput"
    ).ap()
    for nm in ("w2p", "w3p"):
        t[nm] = nc.dram_tensor(
            nm, [M_TILES, P, M_TILES, P], f8, kind="ExternalInput"
        ).ap()
    t["w4d"] = nc.dram_tensor(
        "w4d", [M_TILES // 2, P, 2, W4M], f8, kind="ExternalInput"
    ).ap()
    t["b4t"] = nc.dram_tensor("b4t", [10, 1], f32, kind="ExternalInput").ap()
    for i in (1, 2, 3):
        t[f"s{i}"] = nc.dram_tensor(f"s{i}", [P, M_TILES], f32, kind="ExternalInput").ap()
        t[f"c{i}"] = nc.dram_tensor(f"c{i}", [P, M_TILES], f32, kind="ExternalInput").ap()
    t["out"] = nc.dram_tensor("out", [B, 10], f32, kind="ExternalOutput").ap()

    from contextlib import ExitStack

    with tile.TileContext(nc) as tc, ExitStack() as ctx:
        consts = ctx.enter_context(tc.tile_pool(name="consts", bufs=1))
        xpool = ctx.enter_context(tc.tile_pool(name="x", bufs=1))
        hpool = ctx.enter_context(tc.tile_pool(name="h", bufs=2))
        w1pool = ctx.enter_context(tc.tile_pool(name="w1", bufs=5))
        w1tpool = ctx.enter_context(tc.tile_pool(name="w1t", bufs=3))
        wpool = ctx.enter_context(tc.tile_pool(name="w", bufs=4))
        pspool = ctx.enter_context(tc.tile_pool(name="ps", bufs=8, space="PSUM"))
        small = ctx.enter_context(tc.tile_pool(name="small", bufs=4))

        # ---- one-time loads ----
        # w1[0] first on the sync queue (split so the very first matmul only
        # waits for one k-slice), x chunks on gpsimd+vector in consumption
        # order, consts on the scalar queue.
        w1tiles = {}

        def fetch_w1(m):
            wf = w1pool.tile([P, KF // 2, P], f16, tag="w1f")
            if m == 0:
                nc.sync.dma_start(wf[:, 0:1, :], t["w1f"][m][:, 0:1, :])
                nc.sync.dma_start(wf[:, 1 : KF // 2, :], t["w1f"][m][:, 1 : KF // 2, :])
            else:
                nc.sync.dma_start(wf[:], t["w1f"][m])
            w1tiles[m] = wf

        fetch_w1(0)
        bn = []
        s_t = consts.tile([P, M_TILES], f32, tag="s1")
        nc.sync.dma_start(s_t[:], t["s1"][:])
        c_t = consts.tile([P, M_TILES], f32, tag="c1")
        nc.sync.dma_start(c_t[:], t["c1"][:])
        bn.append((s_t, c_t))
        xp_t = xpool.tile([P, KF, B], f16, tag="xp")
        xt_t = xpool.tile([P, B], f16, tag="xt")
        for n in range(NB):
            sl = slice(n * 512, (n + 1) * 512)
            for j in range(KF):
                q = nc.gpsimd if j % 2 == 0 else nc.scalar
                q.dma_start(xp_t[:, j, sl], t["xp"][:, j, sl])
            nc.scalar.dma_start(xt_t[:, sl], t["xt"][:, sl])
        for i in (2, 3):
            s_t = consts.tile([P, M_TILES], f32, tag=f"s{i}")
            nc.scalar.dma_start(s_t[:], t[f"s{i}"][:])
            c_t = consts.tile([P, M_TILES], f32, tag=f"c{i}")
            nc.scalar.dma_start(c_t[:], t[f"c{i}"][:])
            bn.append((s_t, c_t))
        w4sb = consts.tile([P, M_TILES // 2, 2, W4M], f8, tag="w4")
        nc.scalar.dma_start(w4sb[:], t["w4d"].rearrange("k p o m -> p k o m"))
        b4sb = consts.tile([10, 1], f32, tag="b4")
        nc.scalar.dma_start(b4sb[:], t["b4t"][:])
        ident10 = consts.tile([10, 10], f32, tag="ident")
        make_identity(nc, ident10[:])

        # ---- layer 1: 2 fp16 planes of x; per psum tile 12 full matmuls
        # (x k-tiles 0-5 = hi, 6-11 = lo, sharing the 6 weight slices) plus a
        # 32-row tail (both planes' cols 768..783).  Tiles go in groups of 4
        # (2 m-tiles x 2 n-chunks); the 4 tails run concurrently in separate
        # 32-row PE row-groups via tile_position.
        s_t, c_t = bn[0]
        h1 = hpool.tile([P, M_TILES, B], f8, tag="h")
        for g in range(M_TILES // 2):
            m0, m1 = 2 * g, 2 * g + 1
            for m in (m0, m1):
                if m not in w1tiles:
                    fetch_w1(m)
            wt4 = w1tpool.tile([P, P], f16, tag="w1t4")
            nc.sync.dma_start(wt4[:], t["w1t"][g])
            group = [(m0, 0), (m1, 0), (m0, 1), (m1, 1)]
            pss = []
            for m, n in group:
                wf = w1tiles[m]
                sl = slice(n * 512, (n + 1) * 512)
                ps = pspool.tile([P, 512], f32, tag="ps")
                for jj in range(KF // 2):
                    nc.tensor.matmul(
                        ps[:], wf[:, jj, :], xp_t[:, jj, sl], start=(jj == 0), stop=False
                    )
                    nc.tensor.matmul(
                        ps[:], wf[:, jj, :], xp_t[:, jj + KF // 2, sl],
                        start=False, stop=False,
                    )
                pss.append((ps, m, sl))
            for q, (ps, m, sl) in enumerate(pss):
                nc.tensor.matmul(
                    ps[:],
                    wt4[32 * q : 32 * q + 32, :],
                    xt_t[32 * q : 32 * q + 32, sl],
                    start=False,
                    stop=True,
                    tile_position=(32 * q, 0),
                )
            for ps, m, sl in pss:
                nc.scalar.activation(
                    h1[:, m, sl],
                    ps[:],
                    af.Sign,
                    bias=c_t[:, m : m + 1],
                    scale=s_t[:, m : m + 1],
                )

        # ---- layers 2 and 3: exact +-1 fp8 DoubleRow matmuls ----
        hin = h1
        for li, wname in ((1, "w2p"), (2, "w3p")):
            s_t, c_t = bn[li]
            hout = hpool.tile([P, M_TILES, B], f8, tag="h")
            for m in range(M_TILES):
                wt = wpool.tile([P, M_TILES, P], f8, tag="w")
                (nc.sync if m % 2 == 0 else nc.gpsimd).dma_start(wt[:], t[wname][m])
                for n in range(NB):
                    ps = pspool.tile([P, 512], f32, tag="ps")
                    for k2 in range(M_TILES // 2):
                        nc.tensor.matmul(
                            ps[:],
                            wt[:, 2 * k2 : 2 * k2 + 2, :],
                            hin[:, 2 * k2 : 2 * k2 + 2, n * 512 : (n + 1) * 512],
                            start=(k2 == 0),
                            stop=(k2 == M_TILES // 2 - 1),
                            perf_mode=mybir.MatmulPerfMode.DoubleRow,
                        )
                    nc.scalar.activation(
                        hout[:, m, n * 512 : (n + 1) * 512],
                        ps[:],
                        af.Sign,
                        bias=c_t[:, m : m + 1],
                        scale=s_t[:, m : m + 1],
                    )
            hin = hout
        h3 = hin

        # ---- layer 4 + log_softmax ----
        # Pre-stage the Ln act-table while the L4 matmuls run (all Sign ops
        # are done by now, so nothing evicts it before the tail Ln).
        warm = small.tile([1, 1], f32, tag="warm")
        nc.vector.memset(warm[:], 1.0)
        warmo = small.tile([1, 1], f32, tag="warmo")
        nc.scalar.activation(warmo[:], warm[:], af.Ln)
        se_all = small.tile([P, BCH], f32, tag="se_all")
        otp_all = small.tile([P, BCH, 10], f32, tag="otp_all")
        # 4 fp8 planes of 64*w4.T at psum partitions 0/32/64/96; DoubleRow
        # over 24 k-pairs.  lgb = sum of planes + 64*b4; the (1/64) rescale
        # rides the transpose identity.
        lgbs = []
        for n in range(NB):
            sl = slice(n * 512, (n + 1) * 512)
            ps20 = pspool.tile([P, 512], f32, tag="ps")
            for k2 in range(M_TILES // 2):
                nc.tensor.matmul(
                    ps20[0:W4M, :],
                    w4sb[:, k2, :, :],
                    h3[:, 2 * k2 : 2 * k2 + 2, sl],
                    start=(k2 == 0),
                    stop=(k2 == M_TILES // 2 - 1),
                    perf_mode=mybir.MatmulPerfMode.DoubleRow,
                )
            # DVE may read at most one PSUM operand per op: chain the 4 plane
            # rows through SBUF.  lgb = (sum(planes) + 64*b4) / 64, exact.
            t1 = small.tile([10, 512], f32, tag="t1")
            nc.vector.tensor_scalar_add(t1[:], ps20[0:10, :], b4sb[:])
            t2 = small.tile([10, 512], f32, tag="t2")
            nc.vector.tensor_add(t2[:], t1[:], ps20[32:42, :])
            t3 = small.tile([10, 512], f32, tag="t3")
            nc.vector.tensor_add(t3[:], t2[:], ps20[64:74, :])
            t4 = small.tile([10, 512], f32, tag="t4")
            nc.vector.tensor_add(t4[:], t3[:], ps20[96:106, :])
            lgb = small.tile([10, 512], f32, tag="lgb")
            nc.vector.tensor_scalar_mul(lgb[:], t4[:], 1.0 / W4SCALE)
            lgbs.append(lgb)
        # pass A per 128-row block: transpose (with 1/64 rescale), max,
        # shift, exp(+sum)
        for n in range(NB):
            lgb = lgbs[n]
            for bi in range(4):
                pstt = pspool.tile([P, 512], f32, tag="ps")
                pst = pstt[:, 0:10]
                nc.tensor.transpose(pst, lgb[:, bi * P : (bi + 1) * P], ident10[:])
                nmx = small.tile([P, 1], f32, tag="nmx")
                nc.vector.reduce_max(nmx[:], pst, axis=mybir.AxisListType.X, negate=True)
                ci = n * 4 + bi
                nc.vector.tensor_scalar_add(otp_all[:, ci, :], pst, nmx[:])
                ex = small.tile([P, 10], f32, tag="ex")
                nc.scalar.activation(
                    ex[:],
                    pst,
                    af.Exp,
                    bias=nmx[:],
                    scale=1.0,
                    accum_out=se_all[:, ci : ci + 1],
                )
        # pass B: one batched Ln, one broadcast subtract, one store
        ls_all = small.tile([P, BCH], f32, tag="ls_all")
        nc.scalar.activation(ls_all[:], se_all[:], af.Ln)
        acc = small.tile([P, BCH, 10], f32, tag="acc")
        nc.vector.tensor_sub(
            acc[:],
            otp_all[:],
            ls_all[:].rearrange("p (b o) -> p b o", o=1).broadcast_to([P, BCH, 10]),
        )
        nc.sync.dma_start(t["out"].rearrange("(b p) n -> p b n", p=P), acc[:])

    # Serve Sign/Exp/Ln from one activation-table set if a single set covers
    # all three (natural_log_exp_and_others does on TRN2): the ACT table RAM
    # holds one set at a time, so this removes the ~2.7us Ln table reload
    # from the critical path at the end of the kernel.
    import concourse.bacc as bacc_mod

    orig_tables = bacc_mod.get_activation_tables
    try:
        tables = orig_tables(nc.m.arch)
        need = {af.Sign, af.Exp, af.Ln}
        good = next((k for k, v in tables.items() if need <= v), None)
        if good is not None:
            filtered = {k: (v if k == good else set()) for k, v in tables.items()}
            bacc_mod.get_activation_tables = lambda arch, _f=filtered: _f
        nc.compile()
    finally:
        bacc_mod.get_activation_tables = orig_tables
    return nc


def prepare_in_maps(inputs):
    """Host-side packing: binarize weights, fold BN, split/shard x."""
    x = np.asarray(inputs["x"], np.float32).reshape(-1, K1)

    wb1 = _binarize(np.asarray(inputs["w1"], np.float32))  # [6144, 784]
    # 6 shared weight slices (cols 0..767) serve both planes' k-tiles; tail:
    # cols 768..783 for both planes stacked into 32 rows, replicated to the
    # 4 PE row-groups (one per psum tile of a group).
    w1f = np.ascontiguousarray(
        wb1[:, :768].reshape(M_TILES, P, KF // 2, P).transpose(0, 3, 2, 1)
    ).astype(_FP16)
    wt16 = wb1[:, 768:784]
    wtl = np.concatenate([wt16, wt16], axis=1)  # [6144, 32]
    tailT = np.ascontiguousarray(
        wtl.reshape(M_TILES, P, KT).transpose(0, 2, 1)
    ).astype(_FP16)  # [48, 32, 128]
    w1t = np.zeros((M_TILES // 2, P, P), _FP16)
    for g in range(M_TILES // 2):
        w1t[g, 0:32] = tailT[2 * g]
        w1t[g, 32:64] = tailT[2 * g + 1]
        w1t[g, 64:96] = tailT[2 * g]
        w1t[g, 96:128] = tailT[2 * g + 1]

    w2p = _pack_weight(_binarize(np.asarray(inputs["w2"], np.float32)), H, _FP8)
    w3p = _pack_weight(_binarize(np.asarray(inputs["w3"], np.float32)), H, _FP8)

    # w4: scale by 64, peel 4 exact fp8 planes, stack at psum cols 0/32/64/96
    w4 = np.asarray(inputs["w4"], np.float32)
    b4 = np.asarray(inputs["b4"], np.float32)
    w4T = np.ascontiguousarray(w4.T) * np.float32(W4SCALE)  # [6144, 10]
    p4 = np.zeros((H, W4M), _FP8)
    r = w4T.copy()
    for i in range(NPL):
        pl = r.astype(_FP8)
        p4[:, 32 * i : 32 * i + 10] = pl
        r = r - pl.astype(np.float32)
    w4d = np.ascontiguousarray(
        p4.reshape(M_TILES // 2, 2, P, W4M).transpose(0, 2, 1, 3)
    )
    b4t = np.ascontiguousarray((b4 * np.float32(W4SCALE)).reshape(10, 1))

    sc = {}
    for i in (1, 2, 3):
        g = np.asarray(inputs[f"g{i}"], np.float32)
        be = np.asarray(inputs[f"be{i}"], np.float32)
        m = np.asarray(inputs[f"m{i}"], np.float32)
        v = np.asarray(inputs[f"v{i}"], np.float32)
        b = np.asarray(inputs[f"b{i}"], np.float32)
        s = g / np.sqrt(v + np.float32(EPS))
        c = (b - m) * s + be
        sc[f"s{i}"] = np.ascontiguousarray(s.reshape(M_TILES, P).T)
        sc[f"c{i}"] = np.ascontiguousarray(c.reshape(M_TILES, P).T)

    # x: 2-way fp16 split (PE keeps fp16 denormals), pack into 12 full
    # k-tiles (hi/lo cols 0..767) + one 32-row tail (cols 768..783)
    x_hi = x.astype(_FP16)
    x_lo = (x - x_hi.astype(np.float32)).astype(_FP16)

    in_maps = []
    for core in range(N_CORES):
        sl = slice(core * B, (core + 1) * B)
        im = {
            "w1f": w1f,
            "w1t": w1t,
            "w2p": w2p,
            "w3p": w3p,
            "w4d": w4d,
            "b4t": b4t,
            **sc,
        }
        hi = x_hi[sl]  # [B, 784]
        lo = x_lo[sl]
        xp = np.concatenate(
            [
                hi[:, :768].T.reshape(6, P, B),
                lo[:, :768].T.reshape(6, P, B),
            ],
            axis=0,
        ).transpose(1, 0, 2)  # [128, 12, B]
        im["xp"] = np.ascontiguousarray(xp)
        xt = np.concatenate([hi[:, 768:784].T, lo[:, 768:784].T], axis=0)  # [32, B]
        im["xt"] = np.ascontiguousarray(np.tile(xt, (4, 1)))  # [128, B]
        in_maps.append(im)
    return in_maps


_NC_CACHE = []


def kernel(**inputs):
    import time

    from concourse.bass_utils import run_bass_kernel_spmd

    if not _NC_CACHE:
        _NC_CACHE.append(build_nc())
    nc = _NC_CACHE[0]

    in_maps = prepare_in_maps(inputs)
    last_err = None
    for attempt in range(3):
        try:
            res = run_bass_kernel_spmd(nc, in_maps, core_ids=list(range(N_CORES)))
            return np.concatenate([r["out"] for r in res.results], axis=0)
        except Exception as e:  # transient device errors (e.g. NRT exec unit)
            last_err = e
            time.sleep(5 * (attempt + 1))
    raise last_err


# revision 32
# speedup vs baseline: 1.0296x; 1.0035x over previous
"""Trainium2 Bass kernel for the binarized MLP (BNN) problem.

Network (eval mode):
  h1 = sign(bn1(x @ sign(w1).T + b1))        x: [8192, 784]
  h2 = sign(bn2(h1 @ sign(w2).T + b2))       hidden: 6144
  h3 = sign(bn3(h2 @ sign(w3).T + b3))
  out = log_softmax(h3 @ w4.T + b4)          out: [8192, 10]
(clip(-1,1) before sign does not change sign, so it is dropped.)

Strategy:
  * Data-parallel over the batch: 8 cores x 1024 rows, no collectives.
  * All activations live transposed in SBUF as hT[H, B] so each layer's
    output feeds the next layer's matmul rhs directly (zero transposes).
  * BN + bias + clip + binarize folds to sign(h*s + c) with
    s = g*rsqrt(v+eps), c = (b - m)*s + be  -> one scalar-engine
    activation (Sign) per psum tile with per-partition scale/bias.
  * Layer 1 (real-valued x, contraction 784): x is split into 2 fp16
    planes (hi/lo) so the fp16 matmuls reproduce fp32 precision
    (residual 2^-23; the PE keeps fp16 denormals, verified on HW);
    weights are exact +-1 in fp16.  The two planes are packed into
    12 full 128-row k-tiles (2*768 rows) plus one 32-row tail tile
    (2*16 rows), so each psum tile needs 13 matmuls instead of 14.
  * Layers 2/3 (+-1 x +-1, contraction 6144): fp8e4m3 with DoubleRow
    perf mode - products and fp32 PSUM accumulation are exact.
  * Layer 4: w4.T is scaled by 64 and decomposed into 4 exact fp8
    planes (e4m3 peel-off), stacked at psum partitions 0/32/64/96 of a
    [106, 512] DoubleRow matmul against the fp8 h3 - halves the
    classifier matmul count vs a bf16 hi/lo scheme.  DVE sums the 4
    plane rows + 64*b4, the PE transposes [10,128] blocks back with a
    (1/64)-scaled identity, and log_softmax runs per [128, 10] tile
    (reduce_max, Exp with accumulate, Ln, subtract).
"""

import numpy as np
import ml_dtypes

H = 6144
B_TOTAL = 8192
N_CORES = 8
B = B_TOTAL // N_CORES  # 1024 rows per core
K1 = 784
EPS = 1e-5
P = 128
M_TILES = H // P  # 48
NB = B // 512  # psum-width chunks per core
BCH = B // P  # 8 output row-chunks per core
KF = 12  # full 128-row k-tiles in layer 1 (2 planes x 768 rows)
KT = 32  # tail k-tile rows (2 planes x 16 rows)
NPL = 4  # fp8 planes for w4
W4M = 112  # psum partitions for layer 4 (planes at 0/32/64/96, padded so the
# DoubleRow weight AP's Ko stride (= W4M fp8 bytes) is 16-byte aligned
W4SCALE = 64.0

_BF16 = ml_dtypes.bfloat16
_FP8 = ml_dtypes.float8_e4m3
_FP16 = np.float16


def _binarize(w):
    return np.where(w >= 0, np.float32(1.0), np.float32(-1.0))


def _pack_weight(wb, kpad, dtype):
    """[Hout, K] +-1 matrix -> [Hout/128, 128, kpad/128, 128] tiles where
    pack[m, p, ko, j] = wb[m*128 + j, ko*128 + p] (lhsT layout)."""
    hout, k = wb.shape
    if k < kpad:
        wb = np.concatenate([wb, np.zeros((hout, kpad - k), np.float32)], axis=1)
    return np.ascontiguousarray(
        wb.reshape(hout // P, P, kpad // P, P).transpose(0, 3, 2, 1)
    ).astype(dtype)


def build_nc():
    """Build the (single-program, run-on-8-cores) Bass kernel."""
    import concourse.tile as tile
    import concourse.mybir as mybir
    from concourse import bacc
    from concourse.masks import make_identity

    af = mybir.ActivationFunctionType
    f32 = mybir.dt.float32
    f16 = mybir.dt.float16
    f8 = mybir.dt.float8e4

    nc = bacc.Bacc(
        "TRN2",
        target_bir_lowering=False,
        debug=False,
        enable_asserts=False,
        num_devices=N_CORES,
    )

    t = {}
    t["xp"] = nc.dram_tensor("xp", [P, KF, B], f16, kind="ExternalInput").ap()
    t["xt"] = nc.dram_tensor("xt", [P, B], f16, kind="ExternalInput").ap()
    t["w1f"] = nc.dram_tensor(
        "w1f", [M_TILES, P, KF // 2, P], f16, kind="ExternalInput"
    ).ap()
    t["w1t"] = nc.dram_tensor(
        "w1t", [M_TILES // 4, P, P], f16, kind="ExternalInput"
    ).ap()
    for nm in ("w2p", "w3p"):
        t[nm] = nc.dram_tensor(
            nm, [M_TILES, P, M_TILES, P], f8, kind="ExternalInput"
        ).ap()
    t["w4d"] = nc.dram_tensor(
        "w4d", [M_TILES // 2, P, 2, W4M], f8, kind="ExternalInput"
    ).ap()
    t["b4t"] = nc.dram_tensor("b4t", [10, 1], f32, kind="ExternalInput").ap()
    for i in (1, 2, 3):
        t[f"s{i}"] = nc.dram_tensor(f"s{i}", [P, M_TILES], f32, kind="ExternalInput").ap()
        t[f"c{i}"] = nc.dram_tensor(f"c{i}", [P, M_TILES], f32, kind="ExternalInput").ap()
    t["out"] = nc.dram_tensor("out", [B, 10], f32, kind="ExternalOutput").ap()

    from contextlib import ExitStack

    with tile.TileContext(nc) as tc, ExitStack() as ctx:
        consts = ctx.enter_context(tc.tile_pool(name="consts", bufs=1))
        xpool = ctx.enter_context(tc.tile_pool(name="x", bufs=1))
        hpool = ctx.enter_context(tc.tile_pool(name="h", bufs=2))
        w1pool = ctx.enter_context(tc.tile_pool(name="w1", bufs=5))
        w1tpool = ctx.enter_context(tc.tile_pool(name="w1t", bufs=3))
        wpool = ctx.enter_context(tc.tile_pool(name="w", bufs=4))
        pspool = ctx.enter_context(tc.tile_pool(name="ps", bufs=8, space="PSUM"))
        small = ctx.enter_context(tc.tile_pool(name="small", bufs=4))

        # ---- one-time loads ----
        # w1[0] first on the sync queue (split so the very first matmul only
        # waits for one k-slice), x chunks on gpsimd+vector in consumption
        # order, consts on the scalar queue.
        w1tiles = {}

        def fetch_w1(m):
            wf = w1pool.tile([P, KF // 2, P], f16, tag="w1f")
            if m == 0:
                nc.sync.dma_start(wf[:, 0:1, :], t["w1f"][m][:, 0:1, :])
                nc.sync.dma_start(wf[:, 1 : KF // 2, :], t["w1f"][m][:, 1 : KF // 2, :])
            else:
                nc.sync.dma_start(wf[:], t["w1f"][m])
            w1tiles[m] = wf

        fetch_w1(0)
        bn = []
        s_t = consts.tile([P, M_TILES], f32, tag="s1")
        nc.sync.dma_start(s_t[:], t["s1"][:])
        c_t = consts.tile([P, M_TILES], f32, tag="c1")
        nc.sync.dma_start(c_t[:], t["c1"][:])
        bn.append((s_t, c_t))
        xp_t = xpool.tile([P, KF, B], f16, tag="xp")
        xt_t = xpool.tile([P, B], f16, tag="xt")
        # x chunks in matmul-consumption order (hi/lo interleaved), split
        # over the gpsimd and scalar DMA queues
        corder = [jj + h * (KF // 2) for jj in range(KF // 2) for h in (0, 1)]
        for n in range(NB):
            sl = slice(n * 512, (n + 1) * 512)
            for pos, j in enumerate(corder):
                q = nc.gpsimd if pos % 2 == 0 else nc.scalar
                q.dma_start(xp_t[:, j, sl], t["xp"][:, j, sl])
            nc.scalar.dma_start(xt_t[:, sl], t["xt"][:, sl])
        for i in (2, 3):
            s_t = consts.tile([P, M_TILES], f32, tag=f"s{i}")
            nc.scalar.dma_start(s_t[:], t[f"s{i}"][:])
            c_t = consts.tile([P, M_TILES], f32, tag=f"c{i}")
            nc.scalar.dma_start(c_t[:], t[f"c{i}"][:])
            bn.append((s_t, c_t))
        w4sb = consts.tile([P, M_TILES // 2, 2, W4M], f8, tag="w4")
        nc.scalar.dma_start(w4sb[:], t["w4d"].rearrange("k p o m -> p k o m"))
        b4sb = consts.tile([10, 1], f32, tag="b4")
        nc.scalar.dma_start(b4sb[:], t["b4t"][:])
        ident10 = consts.tile([10, 10], f32, tag="ident")
        make_identity(nc, ident10[:])

        # ---- layer 1: 2 fp16 planes of x; per psum tile 12 full matmuls
        # (x k-tiles 0-5 = hi, 6-11 = lo, sharing the 6 weight slices) plus a
        # 32-row tail (both planes' cols 768..783).  Tiles go in groups of 4
        # (2 m-tiles x 2 n-chunks); the 4 tails run concurrently in separate
        # 32-row PE row-groups via tile_position.
        s_t, c_t = bn[0]
        h1 = hpool.tile([P, M_TILES, B], f8, tag="h")
        # two phases (all n=0 tiles, then all n=1) so the n=1 x chunks have
        # the whole first phase to arrive; groups of 4 m-tiles whose 32-row
        # tails run concurrently in the 4 PE row-groups.  w1f tiles are
        # re-fetched in phase 2 (cheaper than holding all 48 in SBUF).
        for n in range(NB):
            sl = slice(n * 512, (n + 1) * 512)
            if n > 0:
                w1tiles.clear()
            for g in range(M_TILES // 4):
                ms = [4 * g + q for q in range(4)]
                for m in ms:
                    if m not in w1tiles:
                        fetch_w1(m)
                wt4 = w1tpool.tile([P, P], f16, tag="w1t4")
                nc.sync.dma_start(wt4[:], t["w1t"][g])
                pss = []
                for m in ms:
                    wf = w1tiles[m]
                    ps = pspool.tile([P, 512], f32, tag="ps")
                    for jj in range(KF // 2):
                        nc.tensor.matmul(
                            ps[:], wf[:, jj, :], xp_t[:, jj, sl],
                            start=(jj == 0), stop=False,
                        )
                        nc.tensor.matmul(
                            ps[:], wf[:, jj, :], xp_t[:, jj + KF // 2, sl],
                            start=False, stop=False,
                        )
                    pss.append((ps, m))
                for q, (ps, m) in enumerate(pss):
                    nc.tensor.matmul(
                        ps[:],
                        wt4[32 * q : 32 * q + 32, :],
                        xt_t[32 * q : 32 * q + 32, sl],
                        start=False,
                        stop=True,
                        tile_position=(32 * q, 0),
                    )
                for ps, m in pss:
                    nc.scalar.activation(
                        h1[:, m, sl],
                        ps[:],
                        af.Sign,
                        bias=c_t[:, m : m + 1],
                        scale=s_t[:, m : m + 1],
                    )

        # ---- layers 2 and 3: exact +-1 fp8 DoubleRow matmuls ----
        hin = h1
        for li, wname in ((1, "w2p"), (2, "w3p")):
            s_t, c_t = bn[li]
            hout = hpool.tile([P, M_TILES, B], f8, tag="h")
            for m in range(M_TILES):
                wt = wpool.tile([P, M_TILES, P], f8, tag="w")
                (nc.sync if m % 2 == 0 else nc.gpsimd).dma_start(wt[:], t[wname][m])
                for n in range(NB):
                    ps = pspool.tile([P, 512], f32, tag="ps")
                    for k2 in range(M_TILES // 2):
                        nc.tensor.matmul(
                            ps[:],
                            wt[:, 2 * k2 : 2 * k2 + 2, :],
                            hin[:, 2 * k2 : 2 * k2 + 2, n * 512 : (n + 1) * 512],
                            start=(k2 == 0),
                            stop=(k2 == M_TILES // 2 - 1),
                            perf_mode=mybir.MatmulPerfMode.DoubleRow,
                        )
                    nc.scalar.activation(
                        hout[:, m, n * 512 : (n + 1) * 512],
                        ps[:],
                        af.Sign,
                        bias=c_t[:, m : m + 1],
                        scale=s_t[:, m : m + 1],
                    )
            hin = hout
        h3 = hin

        # ---- layer 4 + log_softmax ----
        # Pre-stage the Ln act-table while the L4 matmuls run (all Sign ops
        # are done by now, so nothing evicts it before the tail Ln).
        warm = small.tile([1, 1], f32, tag="warm")
        nc.vector.memset(warm[:], 1.0)
        warmo = small.tile([1, 1], f32, tag="warmo")
        nc.scalar.activation(warmo[:], warm[:], af.Ln)
        se_all = small.tile([P, BCH], f32, tag="se_all")
        otp_all = small.tile([P, BCH, 10], f32, tag="otp_all")
        # 4 fp8 planes of 64*w4.T at psum partitions 0/32/64/96; DoubleRow
        # over 24 k-pairs.  lgb = sum of planes + 64*b4; the (1/64) rescale
        # rides the transpose identity.
        lgbs = []
        for n in range(NB):
            sl = slice(n * 512, (n + 1) * 512)
            ps20 = pspool.tile([P, 512], f32, tag="ps")
            for k2 in range(M_TILES // 2):
                nc.tensor.matmul(
                    ps20[0:W4M, :],
                    w4sb[:, k2, :, :],
                    h3[:, 2 * k2 : 2 * k2 + 2, sl],
                    start=(k2 == 0),
                    stop=(k2 == M_TILES // 2 - 1),
                    perf_mode=mybir.MatmulPerfMode.DoubleRow,
                )
            # DVE may read at most one PSUM operand per op: chain the 4 plane
            # rows through SBUF.  lgb = (sum(planes) + 64*b4) / 64, exact.
            t1 = small.tile([10, 512], f32, tag="t1")
            nc.vector.tensor_scalar_add(t1[:], ps20[0:10, :], b4sb[:])
            t2 = small.tile([10, 512], f32, tag="t2")
            nc.vector.tensor_add(t2[:], t1[:], ps20[32:42, :])
            t3 = small.tile([10, 512], f32, tag="t3")
            nc.vector.tensor_add(t3[:], t2[:], ps20[64:74, :])
            t4 = small.tile([10, 512], f32, tag="t4")
            nc.vector.tensor_add(t4[:], t3[:], ps20[96:106, :])
            lgb = small.tile([10, 512], f32, tag="lgb")
            nc.vector.tensor_scalar_mul(lgb[:], t4[:], 1.0 / W4SCALE)
            lgbs.append(lgb)
        # pass A per 128-row block: transpose (with 1/64 rescale), max,
        # shift, exp(+sum)
        for n in range(NB):
            lgb = lgbs[n]
            for bi in range(4):
                pstt = pspool.tile([P, 512], f32, tag="ps")
                pst = pstt[:, 0:10]
                nc.tensor.transpose(pst, lgb[:, bi * P : (bi + 1) * P], ident10[:])
                nmx = small.tile([P, 1], f32, tag="nmx")
                nc.vector.reduce_max(nmx[:], pst, axis=mybir.AxisListType.X, negate=True)
                ci = n * 4 + bi
                nc.vector.tensor_scalar_add(otp_all[:, ci, :], pst, nmx[:])
                ex = small.tile([P, 10], f32, tag="ex")
                nc.scalar.activation(
                    ex[:],
                    pst,
                    af.Exp,
                    bias=nmx[:],
                    scale=1.0,
                    accum_out=se_all[:, ci : ci + 1],
                )
        # pass B: one batched Ln, one broadcast subtract, one store
        ls_all = small.tile([P, BCH], f32, tag="ls_all")
        nc.scalar.activation(ls_all[:], se_all[:], af.Ln)
        acc = small.tile([P, BCH, 10], f32, tag="acc")
        nc.vector.tensor_sub(
            acc[:],
            otp_all[:],
            ls_all[:].rearrange("p (b o) -> p b o", o=1).broadcast_to([P, BCH, 10]),
        )
        nc.sync.dma_start(t["out"].rearrange("(b p) n -> p b n", p=P), acc[:])

    # Serve Sign/Exp/Ln from one activation-table set if a single set covers
    # all three (natural_log_exp_and_others does on TRN2): the ACT table RAM
    # holds one set at a time, so this removes the ~2.7us Ln table reload
    # from the critical path at the end of the kernel.
    import concourse.bacc as bacc_mod

    orig_tables = bacc_mod.get_activation_tables
    try:
        tables = orig_tables(nc.m.arch)
        need = {af.Sign, af.Exp, af.Ln}
        good = next((k for k, v in tables.items() if need <= v), None)
        if good is not None:
            filtered = {k: (v if k == good else set()) for k, v in tables.items()}
            bacc_mod.get_activation_tables = lambda arch, _f=filtered: _f
        nc.compile()
    finally:
        bacc_mod.get_activation_tables = orig_tables
    return nc


def prepare_in_maps(inputs):
    """Host-side packing: binarize weights, fold BN, split/shard x."""
    x = np.asarray(inputs["x"], np.float32).reshape(-1, K1)

    wb1 = _binarize(np.asarray(inputs["w1"], np.float32))  # [6144, 784]
    # 6 shared weight slices (cols 0..767) serve both planes' k-tiles; tail:
    # cols 768..783 for both planes stacked into 32 rows, replicated to the
    # 4 PE row-groups (one per psum tile of a group).
    w1f = np.ascontiguousarray(
        wb1[:, :768].reshape(M_TILES, P, KF // 2, P).transpose(0, 3, 2, 1)
    ).astype(_FP16)
    wt16 = wb1[:, 768:784]
    wtl = np.concatenate([wt16, wt16], axis=1)  # [6144, 32]
    tailT = np.ascontiguousarray(
        wtl.reshape(M_TILES, P, KT).transpose(0, 2, 1)
    ).astype(_FP16)  # [48, 32, 128]
    w1t = np.zeros((M_TILES // 4, P, P), _FP16)
    for g in range(M_TILES // 4):
        for q in range(4):
            w1t[g, 32 * q : 32 * q + 32] = tailT[4 * g + q]

    w2p = _pack_weight(_binarize(np.asarray(inputs["w2"], np.float32)), H, _FP8)
    w3p = _pack_weight(_binarize(np.asarray(inputs["w3"], np.float32)), H, _FP8)

    # w4: scale by 64, peel 4 exact fp8 planes, stack at psum cols 0/32/64/96
    w4 = np.asarray(inputs["w4"], np.float32)
    b4 = np.asarray(inputs["b4"], np.float32)
    w4T = np.ascontiguousarray(w4.T) * np.float32(W4SCALE)  # [6144, 10]
    p4 = np.zeros((H, W4M), _FP8)
    r = w4T.copy()
    for i in range(NPL):
        pl = r.astype(_FP8)
        p4[:, 32 * i : 32 * i + 10] = pl
        r = r - pl.astype(np.float32)
    w4d = np.ascontiguousarray(
        p4.reshape(M_TILES // 2, 2, P, W4M).transpose(0, 2, 1, 3)
    )
    b4t = np.ascontiguousarray((b4 * np.float32(W4SCALE)).reshape(10, 1))

    sc = {}
    for i in (1, 2, 3):
        g = np.asarray(inputs[f"g{i}"], np.float32)
        be = np.asarray(inputs[f"be{i}"], np.float32)
        m = np.asarray(inputs[f"m{i}"], np.float32)
        v = np.asarray(inputs[f"v{i}"], np.float32)
        b = np.asarray(inputs[f"b{i}"], np.float32)
        s = g / np.sqrt(v + np.float32(EPS))
        c = (b - m) * s + be
        sc[f"s{i}"] = np.ascontiguousarray(s.reshape(M_TILES, P).T)
        sc[f"c{i}"] = np.ascontiguousarray(c.reshape(M_TILES, P).T)

    # x: 2-way fp16 split (PE keeps fp16 denormals), pack into 12 full
    # k-tiles (hi/lo cols 0..767) + one 32-row tail (cols 768..783)
    x_hi = x.astype(_FP16)
    x_lo = (x - x_hi.astype(np.float32)).astype(_FP16)

    in_maps = []
    for core in range(N_CORES):
        sl = slice(core * B, (core + 1) * B)
        im = {
            "w1f": w1f,
            "w1t": w1t,
            "w2p": w2p,
            "w3p": w3p,
            "w4d": w4d,
            "b4t": b4t,
            **sc,
        }
        hi = x_hi[sl]  # [B, 784]
        lo = x_lo[sl]
        xp = np.concatenate(
            [
                hi[:, :768].T.reshape(6, P, B),
                lo[:, :768].T.reshape(6, P, B),
            ],
            axis=0,
        ).transpose(1, 0, 2)  # [128, 12, B]
        im["xp"] = np.ascontiguousarray(xp)
        xt = np.concatenate([hi[:, 768:784].T, lo[:, 768:784].T], axis=0)  # [32, B]
        im["xt"] = np.ascontiguousarray(np.tile(xt, (4, 1)))  # [128, B]
        in_maps.append(im)
    return in_maps


_NC_CACHE = []


def kernel(**inputs):
    import time

    from concourse.bass_utils import run_bass_kernel_spmd

    if not _NC_CACHE:
        _NC_CACHE.append(build_nc())
    nc = _NC_CACHE[0]

    in_maps = prepare_in_maps(inputs)
    last_err = None
    for attempt in range(3):
        try:
            res = run_bass_kernel_spmd(nc, in_maps, core_ids=list(range(N_CORES)))
            return np.concatenate([r["out"] for r in res.results], axis=0)
        except Exception as e:  # transient device errors (e.g. NRT exec unit)
            last_err = e
            time.sleep(5 * (attempt + 1))
    raise last_err


# revision 38
# speedup vs baseline: 1.0315x; 1.0019x over previous
"""Trainium2 Bass kernel for the binarized MLP (BNN) problem.

Network (eval mode):
  h1 = sign(bn1(x @ sign(w1).T + b1))        x: [8192, 784]
  h2 = sign(bn2(h1 @ sign(w2).T + b2))       hidden: 6144
  h3 = sign(bn3(h2 @ sign(w3).T + b3))
  out = log_softmax(h3 @ w4.T + b4)          out: [8192, 10]
(clip(-1,1) before sign does not change sign, so it is dropped.)

Strategy:
  * Data-parallel over the batch: 8 cores x 1024 rows, no collectives.
  * All activations live transposed in SBUF as hT[H, B] so each layer's
    output feeds the next layer's matmul rhs directly (zero transposes).
  * BN + bias + clip + binarize folds to sign(h*s + c) with
    s = g*rsqrt(v+eps), c = (b - m)*s + be  -> one scalar-engine
    activation (Sign) per psum tile with per-partition scale/bias.
  * Layer 1 (real-valued x, contraction 784): x is split into 2 fp16
    planes (hi/lo) so the fp16 matmuls reproduce fp32 precision
    (residual 2^-23; the PE keeps fp16 denormals, verified on HW);
    weights are exact +-1 in fp16.  The two planes are packed into
    12 full 128-row k-tiles (2*768 rows) plus one 32-row tail tile
    (2*16 rows), so each psum tile needs 13 matmuls instead of 14.
  * Layers 2/3 (+-1 x +-1, contraction 6144): fp8e4m3 with DoubleRow
    perf mode - products and fp32 PSUM accumulation are exact.
  * Layer 4: w4.T is scaled by 64 and decomposed into 4 exact fp8
    planes (e4m3 peel-off), stacked at psum partitions 0/32/64/96 of a
    [106, 512] DoubleRow matmul against the fp8 h3 - halves the
    classifier matmul count vs a bf16 hi/lo scheme.  DVE sums the 4
    plane rows + 64*b4, the PE transposes [10,128] blocks back with a
    (1/64)-scaled identity, and log_softmax runs per [128, 10] tile
    (reduce_max, Exp with accumulate, Ln, subtract).
"""

import numpy as np
import ml_dtypes

H = 6144
B_TOTAL = 8192
N_CORES = 8
B = B_TOTAL // N_CORES  # 1024 rows per core
K1 = 784
EPS = 1e-5
P = 128
M_TILES = H // P  # 48
NB = B // 512  # psum-width chunks per core
BCH = B // P  # 8 output row-chunks per core
KF = 12  # full 128-row k-tiles in layer 1 (2 planes x 768 rows)
KT = 32  # tail k-tile rows (2 planes x 16 rows)
NPL = 3  # fp8 planes for w4 (peel-off residual < |w4|*2^-12: ~1e-3 logit err)
W4M = 96  # psum partitions for layer 4 (planes at 0/32/64; multiple of 16 so
# the DoubleRow weight AP's Ko stride (= W4M fp8 bytes) is 16-byte aligned
W4SCALE = 64.0

_BF16 = ml_dtypes.bfloat16
_FP8 = ml_dtypes.float8_e4m3
_FP16 = np.float16


def _binarize(w):
    return np.where(w >= 0, np.float32(1.0), np.float32(-1.0))


def _pack_weight(wb, kpad, dtype):
    """[Hout, K] +-1 matrix -> [Hout/128, 128, kpad/128, 128] tiles where
    pack[m, p, ko, j] = wb[m*128 + j, ko*128 + p] (lhsT layout)."""
    hout, k = wb.shape
    if k < kpad:
        wb = np.concatenate([wb, np.zeros((hout, kpad - k), np.float32)], axis=1)
    return np.ascontiguousarray(
        wb.reshape(hout // P, P, kpad // P, P).transpose(0, 3, 2, 1)
    ).astype(dtype)


def build_nc():
    """Build the (single-program, run-on-8-cores) Bass kernel."""
    import concourse.tile as tile
    import concourse.mybir as mybir
    from concourse import bacc
    from concourse.masks import make_identity

    af = mybir.ActivationFunctionType
    f32 = mybir.dt.float32
    f16 = mybir.dt.float16
    f8 = mybir.dt.float8e4

    nc = bacc.Bacc(
        "TRN2",
        target_bir_lowering=False,
        debug=False,
        enable_asserts=False,
        num_devices=N_CORES,
    )

    t = {}
    t["xp"] = nc.dram_tensor("xp", [P, KF, B], f16, kind="ExternalInput").ap()
    t["xt"] = nc.dram_tensor("xt", [P, B], f16, kind="ExternalInput").ap()
    t["w1f"] = nc.dram_tensor(
        "w1f", [M_TILES, P, KF // 2, P], f8, kind="ExternalInput"
    ).ap()
    t["w1t"] = nc.dram_tensor(
        "w1t", [M_TILES // 4, P, P], f8, kind="ExternalInput"
    ).ap()
    for nm in ("w2p", "w3p"):
        t[nm] = nc.dram_tensor(
            nm, [M_TILES, P, M_TILES, P], f8, kind="ExternalInput"
        ).ap()
    t["w4d"] = nc.dram_tensor(
        "w4d", [M_TILES // 2, P, 2, W4M], f8, kind="ExternalInput"
    ).ap()
    t["b4t"] = nc.dram_tensor("b4t", [10, 1], f32, kind="ExternalInput").ap()
    for i in (1, 2, 3):
        t[f"s{i}"] = nc.dram_tensor(f"s{i}", [P, M_TILES], f32, kind="ExternalInput").ap()
        t[f"c{i}"] = nc.dram_tensor(f"c{i}", [P, M_TILES], f32, kind="ExternalInput").ap()
    t["out"] = nc.dram_tensor("out", [B, 10], f32, kind="ExternalOutput").ap()

    from contextlib import ExitStack

    with tile.TileContext(nc) as tc, ExitStack() as ctx:
        consts = ctx.enter_context(tc.tile_pool(name="consts", bufs=1))
        xpool = ctx.enter_context(tc.tile_pool(name="x", bufs=1))
        hpool = ctx.enter_context(tc.tile_pool(name="h", bufs=2))
        w1pool = ctx.enter_context(tc.tile_pool(name="w1", bufs=5))
        w1tpool = ctx.enter_context(tc.tile_pool(name="w1t", bufs=3))
        wpool = ctx.enter_context(tc.tile_pool(name="w", bufs=4))
        pspool = ctx.enter_context(tc.tile_pool(name="ps", bufs=8, space="PSUM"))
        small = ctx.enter_context(tc.tile_pool(name="small", bufs=4))

        # ---- one-time loads ----
        # w1[0] first on the sync queue (split so the very first matmul only
        # waits for one k-slice), x chunks on gpsimd+vector in consumption
        # order, consts on the scalar queue.
        w1tiles = {}

        def fetch_w1(m):
            wf = w1pool.tile([P, KF // 2, P], f8, tag="w1f")
            if m == 0:
                nc.sync.dma_start(wf[:, 0:1, :], t["w1f"][m][:, 0:1, :])
                nc.sync.dma_start(wf[:, 1 : KF // 2, :], t["w1f"][m][:, 1 : KF // 2, :])
            else:
                nc.sync.dma_start(wf[:], t["w1f"][m])
            w1tiles[m] = wf

        fetch_w1(0)
        bn = []
        s_t = consts.tile([P, M_TILES], f32, tag="s1")
        nc.sync.dma_start(s_t[:], t["s1"][:])
        c_t = consts.tile([P, M_TILES], f32, tag="c1")
        nc.sync.dma_start(c_t[:], t["c1"][:])
        bn.append((s_t, c_t))
        xp_t = xpool.tile([P, KF, B], f16, tag="xp")
        xt_t = xpool.tile([P, B], f16, tag="xt")
        # x chunks in matmul-consumption order (hi/lo interleaved), split
        # over the gpsimd and scalar DMA queues
        corder = [jj + h * (KF // 2) for jj in range(KF // 2) for h in (0, 1)]
        for n in range(NB):
            sl = slice(n * 512, (n + 1) * 512)
            for pos, j in enumerate(corder):
                q = nc.gpsimd if pos % 2 == 0 else nc.scalar
                q.dma_start(xp_t[:, j, sl], t["xp"][:, j, sl])
            nc.scalar.dma_start(xt_t[:, sl], t["xt"][:, sl])
        for i in (2, 3):
            s_t = consts.tile([P, M_TILES], f32, tag=f"s{i}")
            nc.scalar.dma_start(s_t[:], t[f"s{i}"][:])
            c_t = consts.tile([P, M_TILES], f32, tag=f"c{i}")
            nc.scalar.dma_start(c_t[:], t[f"c{i}"][:])
            bn.append((s_t, c_t))
        w4sb = consts.tile([P, M_TILES // 2, 2, W4M], f8, tag="w4")
        nc.scalar.dma_start(w4sb[:], t["w4d"].rearrange("k p o m -> p k o m"))
        b4sb = consts.tile([10, 1], f32, tag="b4")
        nc.scalar.dma_start(b4sb[:], t["b4t"][:])
        ident10 = consts.tile([10, 10], f32, tag="ident")
        make_identity(nc, ident10[:])

        # ---- layer 1: 2 fp16 planes of x; per psum tile 12 full matmuls
        # (x k-tiles 0-5 = hi, 6-11 = lo, sharing the 6 weight slices) plus a
        # 32-row tail (both planes' cols 768..783).  Tiles go in groups of 4
        # (2 m-tiles x 2 n-chunks); the 4 tails run concurrently in separate
        # 32-row PE row-groups via tile_position.
        s_t, c_t = bn[0]
        h1 = hpool.tile([P, M_TILES, B], f8, tag="h")
        # two phases (all n=0 tiles, then all n=1) so the n=1 x chunks have
        # the whole first phase to arrive; groups of 4 m-tiles whose 32-row
        # tails run concurrently in the 4 PE row-groups.  w1f tiles are
        # re-fetched in phase 2 (cheaper than holding all 48 in SBUF).
        for n in range(NB):
            sl = slice(n * 512, (n + 1) * 512)
            if n > 0:
                w1tiles.clear()
            for g in range(M_TILES // 4):
                ms = [4 * g + q for q in range(4)]
                for m in ms:
                    if m not in w1tiles:
                        fetch_w1(m)
                wt4 = w1tpool.tile([P, P], f8, tag="w1t4")
                nc.sync.dma_start(wt4[:], t["w1t"][g])
                pss = []
                for m in ms:
                    wf = w1tiles[m]
                    ps = pspool.tile([P, 512], f32, tag="ps")
                    for jj in range(KF // 2):
                        nc.tensor.matmul(
                            ps[:], wf[:, jj, :], xp_t[:, jj, sl],
                            start=(jj == 0), stop=False,
                        )
                        nc.tensor.matmul(
                            ps[:], wf[:, jj, :], xp_t[:, jj + KF // 2, sl],
                            start=False, stop=False,
                        )
                    pss.append((ps, m))
                for q, (ps, m) in enumerate(pss):
                    nc.tensor.matmul(
                        ps[:],
                        wt4[32 * q : 32 * q + 32, :],
                        xt_t[32 * q : 32 * q + 32, sl],
                        start=False,
                        stop=True,
                        tile_position=(32 * q, 0),
                    )
                for ps, m in pss:
                    nc.scalar.activation(
                        h1[:, m, sl],
                        ps[:],
                        af.Sign,
                        bias=c_t[:, m : m + 1],
                        scale=s_t[:, m : m + 1],
                    )

        # ---- layers 2 and 3: exact +-1 fp8 DoubleRow matmuls ----
        hin = h1
        for li, wname in ((1, "w2p"), (2, "w3p")):
            s_t, c_t = bn[li]
            hout = hpool.tile([P, M_TILES, B], f8, tag="h")
            for m in range(M_TILES):
                wt = wpool.tile([P, M_TILES, P], f8, tag="w")
                (nc.sync if m % 2 == 0 else nc.gpsimd).dma_start(wt[:], t[wname][m])
                for n in range(NB):
                    ps = pspool.tile([P, 512], f32, tag="ps")
                    for k2 in range(M_TILES // 2):
                        nc.tensor.matmul(
                            ps[:],
                            wt[:, 2 * k2 : 2 * k2 + 2, :],
                            hin[:, 2 * k2 : 2 * k2 + 2, n * 512 : (n + 1) * 512],
                            start=(k2 == 0),
                            stop=(k2 == M_TILES // 2 - 1),
                            perf_mode=mybir.MatmulPerfMode.DoubleRow,
                        )
                    nc.scalar.activation(
                        hout[:, m, n * 512 : (n + 1) * 512],
                        ps[:],
                        af.Sign,
                        bias=c_t[:, m : m + 1],
                        scale=s_t[:, m : m + 1],
                    )
            hin = hout
        h3 = hin

        # ---- layer 4 + log_softmax ----
        # Pre-stage the Ln act-table while the L4 matmuls run (all Sign ops
        # are done by now, so nothing evicts it before the tail Ln).
        warm = small.tile([1, 1], f32, tag="warm")
        nc.vector.memset(warm[:], 1.0)
        warmo = small.tile([1, 1], f32, tag="warmo")
        nc.scalar.activation(warmo[:], warm[:], af.Ln)
        se_all = small.tile([P, BCH], f32, tag="se_all")
        otp_all = small.tile([P, BCH, 10], f32, tag="otp_all")
        # 4 fp8 planes of 64*w4.T at psum partitions 0/32/64/96; DoubleRow
        # over 24 k-pairs.  lgb = sum of planes + 64*b4; the (1/64) rescale
        # rides the transpose identity.
        lgbs = []
        for n in range(NB):
            sl = slice(n * 512, (n + 1) * 512)
            ps20 = pspool.tile([P, 512], f32, tag="ps")
            for k2 in range(M_TILES // 2):
                nc.tensor.matmul(
                    ps20[0:W4M, :],
                    w4sb[:, k2, :, :],
                    h3[:, 2 * k2 : 2 * k2 + 2, sl],
                    start=(k2 == 0),
                    stop=(k2 == M_TILES // 2 - 1),
                    perf_mode=mybir.MatmulPerfMode.DoubleRow,
                )
            # DVE may read at most one PSUM operand per op: chain the plane
            # rows through SBUF.  lgb = (sum(planes) + 64*b4) / 64, exact.
            t1 = small.tile([10, 512], f32, tag="t1")
            nc.vector.tensor_scalar_add(t1[:], ps20[0:10, :], b4sb[:])
            t2 = small.tile([10, 512], f32, tag="t2")
            nc.vector.tensor_add(t2[:], t1[:], ps20[32:42, :])
            t3 = small.tile([10, 512], f32, tag="t3")
            nc.vector.tensor_add(t3[:], t2[:], ps20[64:74, :])
            lgb = small.tile([10, 512], f32, tag="lgb")
            nc.vector.tensor_scalar_mul(lgb[:], t3[:], 1.0 / W4SCALE)
            lgbs.append(lgb)
        # pass A per 128-row block: transpose (with 1/64 rescale), max,
        # shift, exp(+sum)
        for n in range(NB):
            lgb = lgbs[n]
            for bi in range(4):
                pstt = pspool.tile([P, 512], f32, tag="ps")
                pst = pstt[:, 0:10]
                nc.tensor.transpose(pst, lgb[:, bi * P : (bi + 1) * P], ident10[:])
                nmx = small.tile([P, 1], f32, tag="nmx")
                nc.vector.reduce_max(nmx[:], pst, axis=mybir.AxisListType.X, negate=True)
                ci = n * 4 + bi
                nc.vector.tensor_scalar_add(otp_all[:, ci, :], pst, nmx[:])
                ex = small.tile([P, 10], f32, tag="ex")
                nc.scalar.activation(
                    ex[:],
                    pst,
                    af.Exp,
                    bias=nmx[:],
                    scale=1.0,
                    accum_out=se_all[:, ci : ci + 1],
                )
        # pass B: one batched Ln, one broadcast subtract, one store
        ls_all = small.tile([P, BCH], f32, tag="ls_all")
        nc.scalar.activation(ls_all[:], se_all[:], af.Ln)
        acc = small.tile([P, BCH, 10], f32, tag="acc")
        nc.vector.tensor_sub(
            acc[:],
            otp_all[:],
            ls_all[:].rearrange("p (b o) -> p b o", o=1).broadcast_to([P, BCH, 10]),
        )
        nc.sync.dma_start(t["out"].rearrange("(b p) n -> p b n", p=P), acc[:])

    # Serve Sign/Exp/Ln from one activation-table set if a single set covers
    # all three (natural_log_exp_and_others does on TRN2): the ACT table RAM
    # holds one set at a time, so this removes the ~2.7us Ln table reload
    # from the critical path at the end of the kernel.
    import concourse.bacc as bacc_mod

    orig_tables = bacc_mod.get_activation_tables
    try:
        tables = orig_tables(nc.m.arch)
        need = {af.Sign, af.Exp, af.Ln}
        good = next((k for k, v in tables.items() if need <= v), None)
        if good is not None:
            filtered = {k: (v if k == good else set()) for k, v in tables.items()}
            bacc_mod.get_activation_tables = lambda arch, _f=filtered: _f
        nc.compile()
    finally:
        bacc_mod.get_activation_tables = orig_tables
    return nc


def prepare_in_maps(inputs):
    """Host-side packing: binarize weights, fold BN, split/shard x."""
    x = np.asarray(inputs["x"], np.float32).reshape(-1, K1)

    wb1 = _binarize(np.asarray(inputs["w1"], np.float32))  # [6144, 784]
    # 6 shared weight slices (cols 0..767) serve both planes' k-tiles; tail:
    # cols 768..783 for both planes stacked into 32 rows, replicated to the
    # 4 PE row-groups (one per psum tile of a group).
    w1f = np.ascontiguousarray(
        wb1[:, :768].reshape(M_TILES, P, KF // 2, P).transpose(0, 3, 2, 1)
    ).astype(_FP8)
    wt16 = wb1[:, 768:784]
    wtl = np.concatenate([wt16, wt16], axis=1)  # [6144, 32]
    tailT = np.ascontiguousarray(
        wtl.reshape(M_TILES, P, KT).transpose(0, 2, 1)
    ).astype(_FP8)  # [48, 32, 128]
    w1t = np.zeros((M_TILES // 4, P, P), _FP8)
    for g in range(M_TILES // 4):
        for q in range(4):
            w1t[g, 32 * q : 32 * q + 32] = tailT[4 * g + q]

    w2p = _pack_weight(_binarize(np.asarray(inputs["w2"], np.float32)), H, _FP8)
    w3p = _pack_weight(_binarize(np.asarray(inputs["w3"], np.float32)), H, _FP8)

    # w4: scale by 64, peel 4 exact fp8 planes, stack at psum cols 0/32/64/96
    w4 = np.asarray(inputs["w4"], np.float32)
    b4 = np.asarray(inputs["b4"], np.float32)
    w4T = np.ascontiguousarray(w4.T) * np.float32(W4SCALE)  # [6144, 10]
    p4 = np.zeros((H, W4M), _FP8)
    r = w4T.copy()
    for i in range(NPL):
        pl = r.astype(_FP8)
        p4[:, 32 * i : 32 * i + 10] = pl
        r = r - pl.astype(np.float32)
    w4d = np.ascontiguousarray(
        p4.reshape(M_TILES // 2, 2, P, W4M).transpose(0, 2, 1, 3)
    )
    b4t = np.ascontiguousarray((b4 * np.float32(W4SCALE)).reshape(10, 1))

    sc = {}
    for i in (1, 2, 3):
        g = np.asarray(inputs[f"g{i}"], np.float32)
        be = np.asarray(inputs[f"be{i}"], np.float32)
        m = np.asarray(inputs[f"m{i}"], np.float32)
        v = np.asarray(inputs[f"v{i}"], np.float32)
        b = np.asarray(inputs[f"b{i}"], np.float32)
        s = g / np.sqrt(v + np.float32(EPS))
        c = (b - m) * s + be
        sc[f"s{i}"] = np.ascontiguousarray(s.reshape(M_TILES, P).T)
        sc[f"c{i}"] = np.ascontiguousarray(c.reshape(M_TILES, P).T)

    # x: 2-way fp16 split (PE keeps fp16 denormals), pack into 12 full
    # k-tiles (hi/lo cols 0..767) + one 32-row tail (cols 768..783)
    x_hi = x.astype(_FP16)
    x_lo = (x - x_hi.astype(np.float32)).astype(_FP16)

    in_maps = []
    for core in range(N_CORES):
        sl = slice(core * B, (core + 1) * B)
        im = {
            "w1f": w1f,
            "w1t": w1t,
            "w2p": w2p,
            "w3p": w3p,
            "w4d": w4d,
            "b4t": b4t,
            **sc,
        }
        hi = x_hi[sl]  # [B, 784]
        lo = x_lo[sl]
        xp = np.concatenate(
            [
                hi[:, :768].T.reshape(6, P, B),
                lo[:, :768].T.reshape(6, P, B),
            ],
            axis=0,
        ).transpose(1, 0, 2)  # [128, 12, B]
        im["xp"] = np.ascontiguousarray(xp)
        xt = np.concatenate([hi[:, 768:784].T, lo[:, 768:784].T], axis=0)  # [32, B]
        im["xt"] = np.ascontiguousarray(np.tile(xt, (4, 1)))  # [128, B]
        in_maps.append(im)
    return in_maps


_NC_CACHE = []


def kernel(**inputs):
    import time

    from concourse.bass_utils import run_bass_kernel_spmd

    if not _NC_CACHE:
        _NC_CACHE.append(build_nc())
    nc = _NC_CACHE[0]

    in_maps = prepare_in_maps(inputs)
    last_err = None
    for attempt in range(3):
        try:
            res = run_bass_kernel_spmd(nc, in_maps, core_ids=list(range(N_CORES)))
            return np.concatenate([r["out"] for r in res.results], axis=0)
        except Exception as e:  # transient device errors (e.g. NRT exec unit)
            last_err = e
            time.sleep(5 * (attempt + 1))
    raise last_err


# revision 41
# speedup vs baseline: 1.0332x; 1.0017x over previous
"""Trainium2 Bass kernel for the binarized MLP (BNN) problem.

Network (eval mode):
  h1 = sign(bn1(x @ sign(w1).T + b1))        x: [8192, 784]
  h2 = sign(bn2(h1 @ sign(w2).T + b2))       hidden: 6144
  h3 = sign(bn3(h2 @ sign(w3).T + b3))
  out = log_softmax(h3 @ w4.T + b4)          out: [8192, 10]
(clip(-1,1) before sign does not change sign, so it is dropped.)

Strategy:
  * Data-parallel over the batch: 8 cores x 1024 rows, no collectives.
  * All activations live transposed in SBUF as hT[H, B] so each layer's
    output feeds the next layer's matmul rhs directly (zero transposes).
  * BN + bias + clip + binarize folds to sign(h*s + c) with
    s = g*rsqrt(v+eps), c = (b - m)*s + be  -> one scalar-engine
    activation (Sign) per psum tile with per-partition scale/bias.
  * Layer 1 (real-valued x, contraction 784): x is split into 2 fp16
    planes (hi/lo) so the fp16 matmuls reproduce fp32 precision
    (residual 2^-23; the PE keeps fp16 denormals, verified on HW);
    weights are exact +-1 in fp16.  The two planes are packed into
    12 full 128-row k-tiles (2*768 rows) plus one 32-row tail tile
    (2*16 rows), so each psum tile needs 13 matmuls instead of 14.
  * Layers 2/3 (+-1 x +-1, contraction 6144): fp8e4m3 with DoubleRow
    perf mode - products and fp32 PSUM accumulation are exact.
  * Layer 4: w4.T is scaled by 64 and decomposed into 4 exact fp8
    planes (e4m3 peel-off), stacked at psum partitions 0/32/64/96 of a
    [106, 512] DoubleRow matmul against the fp8 h3 - halves the
    classifier matmul count vs a bf16 hi/lo scheme.  DVE sums the 4
    plane rows + 64*b4, the PE transposes [10,128] blocks back with a
    (1/64)-scaled identity, and log_softmax runs per [128, 10] tile
    (reduce_max, Exp with accumulate, Ln, subtract).
"""

import numpy as np
import ml_dtypes

H = 6144
B_TOTAL = 8192
N_CORES = 8
B = B_TOTAL // N_CORES  # 1024 rows per core
K1 = 784
EPS = 1e-5
P = 128
M_TILES = H // P  # 48
NB = B // 512  # psum-width chunks per core
BCH = B // P  # 8 output row-chunks per core
KF = 12  # full 128-row k-tiles in layer 1 (2 planes x 768 rows)
KT = 32  # tail k-tile rows (2 planes x 16 rows)
NPL = 3  # fp8 planes for w4 (peel-off residual < |w4|*2^-12: ~1e-3 logit err)
W4M = 96  # psum partitions for layer 4 (planes at 0/32/64; multiple of 16 so
# the DoubleRow weight AP's Ko stride (= W4M fp8 bytes) is 16-byte aligned
W4SCALE = 64.0

_BF16 = ml_dtypes.bfloat16
_FP8 = ml_dtypes.float8_e4m3
_FP16 = np.float16


def _binarize(w):
    return np.where(w >= 0, np.float32(1.0), np.float32(-1.0))


def _pack_weight(wb, kpad, dtype):
    """[Hout, K] +-1 matrix -> [Hout/128, 128, kpad/128, 128] tiles where
    pack[m, p, ko, j] = wb[m*128 + j, ko*128 + p] (lhsT layout)."""
    hout, k = wb.shape
    if k < kpad:
        wb = np.concatenate([wb, np.zeros((hout, kpad - k), np.float32)], axis=1)
    return np.ascontiguousarray(
        wb.reshape(hout // P, P, kpad // P, P).transpose(0, 3, 2, 1)
    ).astype(dtype)


def build_nc():
    """Build the (single-program, run-on-8-cores) Bass kernel."""
    import concourse.tile as tile
    import concourse.mybir as mybir
    from concourse import bacc
    from concourse.masks import make_identity

    af = mybir.ActivationFunctionType
    f32 = mybir.dt.float32
    f16 = mybir.dt.float16
    f8 = mybir.dt.float8e4

    nc = bacc.Bacc(
        "TRN2",
        target_bir_lowering=False,
        debug=False,
        enable_asserts=False,
        num_devices=N_CORES,
    )

    t = {}
    t["xp"] = nc.dram_tensor("xp", [P, KF, B], f16, kind="ExternalInput").ap()
    t["xt"] = nc.dram_tensor("xt", [P, B], f16, kind="ExternalInput").ap()
    t["w1f"] = nc.dram_tensor(
        "w1f", [M_TILES, P, KF // 2, P], f8, kind="ExternalInput"
    ).ap()
    t["w1t"] = nc.dram_tensor(
        "w1t", [M_TILES // 4, P, P], f8, kind="ExternalInput"
    ).ap()
    for nm in ("w2p", "w3p"):
        t[nm] = nc.dram_tensor(
            nm, [M_TILES, P, M_TILES, P], f8, kind="ExternalInput"
        ).ap()
    t["w4d"] = nc.dram_tensor(
        "w4d", [M_TILES // 2, P, 2, W4M], f8, kind="ExternalInput"
    ).ap()
    t["b4t"] = nc.dram_tensor("b4t", [10, 1], f32, kind="ExternalInput").ap()
    for i in (1, 2, 3):
        t[f"s{i}"] = nc.dram_tensor(f"s{i}", [P, M_TILES], f32, kind="ExternalInput").ap()
        t[f"c{i}"] = nc.dram_tensor(f"c{i}", [P, M_TILES], f32, kind="ExternalInput").ap()
    t["out"] = nc.dram_tensor("out", [B, 10], f32, kind="ExternalOutput").ap()

    from contextlib import ExitStack

    with tile.TileContext(nc) as tc, ExitStack() as ctx:
        consts = ctx.enter_context(tc.tile_pool(name="consts", bufs=1))
        xpool = ctx.enter_context(tc.tile_pool(name="x", bufs=1))
        hpool = ctx.enter_context(tc.tile_pool(name="h", bufs=2))
        w1pool = ctx.enter_context(tc.tile_pool(name="w1", bufs=5))
        w1tpool = ctx.enter_context(tc.tile_pool(name="w1t", bufs=3))
        wpool = ctx.enter_context(tc.tile_pool(name="w", bufs=4))
        pspool = ctx.enter_context(tc.tile_pool(name="ps", bufs=8, space="PSUM"))
        small = ctx.enter_context(tc.tile_pool(name="small", bufs=4))

        # ---- one-time loads ----
        # w1[0] first on the sync queue (split so the very first matmul only
        # waits for one k-slice), x chunks on gpsimd+vector in consumption
        # order, consts on the scalar queue.
        w1tiles = {}

        def fetch_w1(m):
            wf = w1pool.tile([P, KF // 2, P], f8, tag="w1f")
            if m == 0:
                nc.sync.dma_start(wf[:, 0:1, :], t["w1f"][m][:, 0:1, :])
                nc.sync.dma_start(wf[:, 1 : KF // 2, :], t["w1f"][m][:, 1 : KF // 2, :])
            else:
                nc.sync.dma_start(wf[:], t["w1f"][m])
            w1tiles[m] = wf

        fetch_w1(0)
        bn = []
        s_t = consts.tile([P, M_TILES], f32, tag="s1")
        nc.sync.dma_start(s_t[:], t["s1"][:])
        c_t = consts.tile([P, M_TILES], f32, tag="c1")
        nc.sync.dma_start(c_t[:], t["c1"][:])
        bn.append((s_t, c_t))
        xp_t = xpool.tile([P, KF, B], f16, tag="xp")
        xt_t = xpool.tile([P, B], f16, tag="xt")
        # n=0 x chunks in matmul-consumption order (hi/lo interleaved) over
        # all three DMA queues; n=1 chunks are emitted between the phases
        # and arrive during phase-1 compute.
        corder = [jj + h * (KF // 2) for jj in range(KF // 2) for h in (0, 1)]
        xqueues = [nc.gpsimd, nc.scalar, nc.sync]
        for i in (2, 3):
            s_t = consts.tile([P, M_TILES], f32, tag=f"s{i}")
            nc.gpsimd.dma_start(s_t[:], t[f"s{i}"][:])
            c_t = consts.tile([P, M_TILES], f32, tag=f"c{i}")
            nc.gpsimd.dma_start(c_t[:], t[f"c{i}"][:])
            bn.append((s_t, c_t))
        w4sb = consts.tile([P, M_TILES // 2, 2, W4M], f8, tag="w4")
        b4sb = consts.tile([10, 1], f32, tag="b4")
        ident10 = consts.tile([10, 10], f32, tag="ident")
        make_identity(nc, ident10[:])

        # ---- layer 1: 2 fp16 planes of x; per psum tile 12 full matmuls
        # (x k-tiles 0-5 = hi, 6-11 = lo, sharing the 6 weight slices) plus a
        # 32-row tail (both planes' cols 768..783).  Tiles go in groups of 4
        # (2 m-tiles x 2 n-chunks); the 4 tails run concurrently in separate
        # 32-row PE row-groups via tile_position.
        s_t, c_t = bn[0]
        h1 = hpool.tile([P, M_TILES, B], f8, tag="h")
        # two phases (all n=0 tiles, then all n=1) so the n=1 x chunks have
        # the whole first phase to arrive; groups of 4 m-tiles whose 32-row
        # tails run concurrently in the 4 PE row-groups.  w1f tiles are
        # re-fetched in phase 2 (cheaper than holding all 48 in SBUF).
        for n in range(NB):
            sl = slice(n * 512, (n + 1) * 512)
            if n == 0:
                for pos, j in enumerate(corder):
                    xqueues[pos % 3].dma_start(xp_t[:, j, sl], t["xp"][:, j, sl])
                nc.gpsimd.dma_start(xt_t[:, sl], t["xt"][:, sl])
                nc.gpsimd.dma_start(
                    w4sb[:], t["w4d"].rearrange("k p o m -> p k o m")
                )
                nc.gpsimd.dma_start(b4sb[:], t["b4t"][:])
            else:
                for pos, j in enumerate(corder):
                    q = nc.gpsimd if pos % 2 == 0 else nc.scalar
                    q.dma_start(xp_t[:, j, sl], t["xp"][:, j, sl])
                nc.gpsimd.dma_start(xt_t[:, sl], t["xt"][:, sl])
                w1tiles.clear()
            for g in range(M_TILES // 4):
                ms = [4 * g + q for q in range(4)]
                for m in ms:
                    if m not in w1tiles:
                        fetch_w1(m)
                wt4 = w1tpool.tile([P, P], f8, tag="w1t4")
                nc.scalar.dma_start(wt4[:], t["w1t"][g])
                pss = []
                for m in ms:
                    wf = w1tiles[m]
                    ps = pspool.tile([P, 512], f32, tag="ps")
                    for jj in range(KF // 2):
                        nc.tensor.matmul(
                            ps[:], wf[:, jj, :], xp_t[:, jj, sl],
                            start=(jj == 0), stop=False,
                        )
                        nc.tensor.matmul(
                            ps[:], wf[:, jj, :], xp_t[:, jj + KF // 2, sl],
                            start=False, stop=False,
                        )
                    pss.append((ps, m))
                for q, (ps, m) in enumerate(pss):
                    nc.tensor.matmul(
                        ps[:],
                        wt4[32 * q : 32 * q + 32, :],
                        xt_t[32 * q : 32 * q + 32, sl],
                        start=False,
                        stop=True,
                        tile_position=(32 * q, 0),
                    )
                for ps, m in pss:
                    nc.scalar.activation(
                        h1[:, m, sl],
                        ps[:],
                        af.Sign,
                        bias=c_t[:, m : m + 1],
                        scale=s_t[:, m : m + 1],
                    )

        # ---- layers 2 and 3: exact +-1 fp8 DoubleRow matmuls ----
        hin = h1
        for li, wname in ((1, "w2p"), (2, "w3p")):
            s_t, c_t = bn[li]
            hout = hpool.tile([P, M_TILES, B], f8, tag="h")
            for m in range(M_TILES):
                wt = wpool.tile([P, M_TILES, P], f8, tag="w")
                (nc.sync if m % 2 == 0 else nc.gpsimd).dma_start(wt[:], t[wname][m])
                for n in range(NB):
                    ps = pspool.tile([P, 512], f32, tag="ps")
                    for k2 in range(M_TILES // 2):
                        nc.tensor.matmul(
                            ps[:],
                            wt[:, 2 * k2 : 2 * k2 + 2, :],
                            hin[:, 2 * k2 : 2 * k2 + 2, n * 512 : (n + 1) * 512],
                            start=(k2 == 0),
                            stop=(k2 == M_TILES // 2 - 1),
                            perf_mode=mybir.MatmulPerfMode.DoubleRow,
                        )
                    nc.scalar.activation(
                        hout[:, m, n * 512 : (n + 1) * 512],
                        ps[:],
                        af.Sign,
                        bias=c_t[:, m : m + 1],
                        scale=s_t[:, m : m + 1],
                    )
            hin = hout
        h3 = hin

        # ---- layer 4 + log_softmax ----
        # Pre-stage the Ln act-table while the L4 matmuls run (all Sign ops
        # are done by now, so nothing evicts it before the tail Ln).
        warm = small.tile([1, 1], f32, tag="warm")
        nc.vector.memset(warm[:], 1.0)
        warmo = small.tile([1, 1], f32, tag="warmo")
        nc.scalar.activation(warmo[:], warm[:], af.Ln)
        se_all = small.tile([P, BCH], f32, tag="se_all")
        otp_all = small.tile([P, BCH, 10], f32, tag="otp_all")
        # 4 fp8 planes of 64*w4.T at psum partitions 0/32/64/96; DoubleRow
        # over 24 k-pairs.  lgb = sum of planes + 64*b4; the (1/64) rescale
        # rides the transpose identity.
        lgbs = []
        for n in range(NB):
            sl = slice(n * 512, (n + 1) * 512)
            ps20 = pspool.tile([P, 512], f32, tag="ps")
            for k2 in range(M_TILES // 2):
                nc.tensor.matmul(
                    ps20[0:W4M, :],
                    w4sb[:, k2, :, :],
                    h3[:, 2 * k2 : 2 * k2 + 2, sl],
                    start=(k2 == 0),
                    stop=(k2 == M_TILES // 2 - 1),
                    perf_mode=mybir.MatmulPerfMode.DoubleRow,
                )
            # DVE may read at most one PSUM operand per op: chain the plane
            # rows through SBUF.  lgb = (sum(planes) + 64*b4) / 64, exact.
            t1 = small.tile([10, 512], f32, tag="t1")
            nc.vector.tensor_scalar_add(t1[:], ps20[0:10, :], b4sb[:])
            t2 = small.tile([10, 512], f32, tag="t2")
            nc.vector.tensor_add(t2[:], t1[:], ps20[32:42, :])
            t3 = small.tile([10, 512], f32, tag="t3")
            nc.vector.tensor_add(t3[:], t2[:], ps20[64:74, :])
            lgb = small.tile([10, 512], f32, tag="lgb")
            nc.vector.tensor_scalar_mul(lgb[:], t3[:], 1.0 / W4SCALE)
            lgbs.append(lgb)
        # pass A per 128-row block: transpose (with 1/64 rescale), max,
        # shift, exp(+sum)
        for n in range(NB):
            lgb = lgbs[n]
            for bi in range(4):
                pstt = pspool.tile([P, 512], f32, tag="ps")
                pst = pstt[:, 0:10]
                nc.tensor.transpose(pst, lgb[:, bi * P : (bi + 1) * P], ident10[:])
                nmx = small.tile([P, 1], f32, tag="nmx")
                nc.vector.reduce_max(nmx[:], pst, axis=mybir.AxisListType.X, negate=True)
                ci = n * 4 + bi
                nc.vector.tensor_scalar_add(otp_all[:, ci, :], pst, nmx[:])
                ex = small.tile([P, 10], f32, tag="ex")
                nc.scalar.activation(
                    ex[:],
                    pst,
                    af.Exp,
                    bias=nmx[:],
                    scale=1.0,
                    accum_out=se_all[:, ci : ci + 1],
                )
        # pass B: one batched Ln, one broadcast subtract, one store
        ls_all = small.tile([P, BCH], f32, tag="ls_all")
        nc.scalar.activation(ls_all[:], se_all[:], af.Ln)
        acc = small.tile([P, BCH, 10], f32, tag="acc")
        nc.vector.tensor_sub(
            acc[:],
            otp_all[:],
            ls_all[:].rearrange("p (b o) -> p b o", o=1).broadcast_to([P, BCH, 10]),
        )
        nc.sync.dma_start(t["out"].rearrange("(b p) n -> p b n", p=P), acc[:])

    # Serve Sign/Exp/Ln from one activation-table set if a single set covers
    # all three (natural_log_exp_and_others does on TRN2): the ACT table RAM
    # holds one set at a time, so this removes the ~2.7us Ln table reload
    # from the critical path at the end of the kernel.
    import concourse.bacc as bacc_mod

    orig_tables = bacc_mod.get_activation_tables
    try:
        tables = orig_tables(nc.m.arch)
        need = {af.Sign, af.Exp, af.Ln}
        good = next((k for k, v in tables.items() if need <= v), None)
        if good is not None:
            filtered = {k: (v if k == good else set()) for k, v in tables.items()}
            bacc_mod.get_activation_tables = lambda arch, _f=filtered: _f
        nc.compile()
    finally:
        bacc_mod.get_activation_tables = orig_tables
    return nc


def prepare_in_maps(inputs):
    """Host-side packing: binarize weights, fold BN, split/shard x."""
    x = np.asarray(inputs["x"], np.float32).reshape(-1, K1)

    wb1 = _binarize(np.asarray(inputs["w1"], np.float32))  # [6144, 784]
    # 6 shared weight slices (cols 0..767) serve both planes' k-tiles; tail:
    # cols 768..783 for both planes stacked into 32 rows, replicated to the
    # 4 PE row-groups (one per psum tile of a group).
    w1f = np.ascontiguousarray(
        wb1[:, :768].reshape(M_TILES, P, KF // 2, P).transpose(0, 3, 2, 1)
    ).astype(_FP8)
    wt16 = wb1[:, 768:784]
    wtl = np.concatenate([wt16, wt16], axis=1)  # [6144, 32]
    tailT = np.ascontiguousarray(
        wtl.reshape(M_TILES, P, KT).transpose(0, 2, 1)
    ).astype(_FP8)  # [48, 32, 128]
    w1t = np.zeros((M_TILES // 4, P, P), _FP8)
    for g in range(M_TILES // 4):
        for q in range(4):
            w1t[g, 32 * q : 32 * q + 32] = tailT[4 * g + q]

    w2p = _pack_weight(_binarize(np.asarray(inputs["w2"], np.float32)), H, _FP8)
    w3p = _pack_weight(_binarize(np.asarray(inputs["w3"], np.float32)), H, _FP8)

    # w4: scale by 64, peel 4 exact fp8 planes, stack at psum cols 0/32/64/96
    w4 = np.asarray(inputs["w4"], np.float32)
    b4 = np.asarray(inputs["b4"], np.float32)
    w4T = np.ascontiguousarray(w4.T) * np.float32(W4SCALE)  # [6144, 10]
    p4 = np.zeros((H, W4M), _FP8)
    r = w4T.copy()
    for i in range(NPL):
        pl = r.astype(_FP8)
        p4[:, 32 * i : 32 * i + 10] = pl
        r = r - pl.astype(np.float32)
    w4d = np.ascontiguousarray(
        p4.reshape(M_TILES // 2, 2, P, W4M).transpose(0, 2, 1, 3)
    )
    b4t = np.ascontiguousarray((b4 * np.float32(W4SCALE)).reshape(10, 1))

    sc = {}
    for i in (1, 2, 3):
        g = np.asarray(inputs[f"g{i}"], np.float32)
        be = np.asarray(inputs[f"be{i}"], np.float32)
        m = np.asarray(inputs[f"m{i}"], np.float32)
        v = np.asarray(inputs[f"v{i}"], np.float32)
        b = np.asarray(inputs[f"b{i}"], np.float32)
        s = g / np.sqrt(v + np.float32(EPS))
        c = (b - m) * s + be
        sc[f"s{i}"] = np.ascontiguousarray(s.reshape(M_TILES, P).T)
        sc[f"c{i}"] = np.ascontiguousarray(c.reshape(M_TILES, P).T)

    # x: 2-way fp16 split (PE keeps fp16 denormals), pack into 12 full
    # k-tiles (hi/lo cols 0..767) + one 32-row tail (cols 768..783)
    x_hi = x.astype(_FP16)
    x_lo = (x - x_hi.astype(np.float32)).astype(_FP16)

    in_maps = []
    for core in range(N_CORES):
        sl = slice(core * B, (core + 1) * B)
        im = {
            "w1f": w1f,
            "w1t": w1t,
            "w2p": w2p,
            "w3p": w3p,
            "w4d": w4d,
            "b4t": b4t,
            **sc,
        }
        hi = x_hi[sl]  # [B, 784]
        lo = x_lo[sl]
        xp = np.concatenate(
            [
                hi[:, :768].T.reshape(6, P, B),
                lo[:, :768].T.reshape(6, P, B),
            ],
            axis=0,
        ).transpose(1, 0, 2)  # [128, 12, B]
        im["xp"] = np.ascontiguousarray(xp)
        xt = np.concatenate([hi[:, 768:784].T, lo[:, 768:784].T], axis=0)  # [32, B]
        im["xt"] = np.ascontiguousarray(np.tile(xt, (4, 1)))  # [128, B]
        in_maps.append(im)
    return in_maps


_NC_CACHE = []


def kernel(**inputs):
    import time

    from concourse.bass_utils import run_bass_kernel_spmd

    if not _NC_CACHE:
        _NC_CACHE.append(build_nc())
    nc = _NC_CACHE[0]

    in_maps = prepare_in_maps(inputs)
    last_err = None
    for attempt in range(3):
        try:
            res = run_bass_kernel_spmd(nc, in_maps, core_ids=list(range(N_CORES)))
            return np.concatenate([r["out"] for r in res.results], axis=0)
        except Exception as e:  # transient device errors (e.g. NRT exec unit)
            last_err = e
            time.sleep(5 * (attempt + 1))
    raise last_err
